# revision 1
# baseline (speedup 1.0000x reference)
"""Trainium2 Bass kernel for nn_Aggregation_Attn.

Computation (per sample i):
    scores[l] = sum_{t,c} q[i,t,c,l] * k[t,i,c]        # contraction over tc=t*c
    s         = softmax(scores)                         # over l
    out[t,c]  = sum_l q[i,t,c,l] * s[l]                 # contraction over l

Shapes: q [32, 64, 256, 64] f32, k [64, 32, 256] f32, out [64, 32, 256] f32.
Data-parallel over n across 8 cores (4 samples/core).

Per-core layout: q sample flattened to [tc=16384, l=64] (contiguous in HBM),
loaded into SBUF as [128 partitions, 8192] (partition p holds tc rows
[p*128, (p+1)*128), each partition row is one contiguous 32 KB HBM chunk).

Phase 1 on TensorE: for each tc0 in 0..127, matmul with stationary k column
k_sb[:, tc0] ([128,1]) and moving q slice [128, 64] -> accumulate psum [1, 64].
Softmax on tiny [1, 64] (DVE reduce_max -> ACT exp with accum -> DVE recip).
Phase 2 on VectorE: q *= s (broadcast over tc), then reduce over l.
"""

import numpy as np

import concourse.bacc as bacc
import concourse.bass as bass
import concourse.mybir as mybir
import concourse.tile as tile
from concourse import bass_utils

N, T, C, L = 32, 64, 256, 64
NCORES = 8
NS = N // NCORES  # samples per core
P = 128

_NC_CACHE = {}

VARIANT = "fp16hw"
G_LOADS = 4  # q load chunks per sample (HW-measured optimum)
DUAL_RING = False  # alternate q-chunk DMAs between SP and ACT HWDGE rings
# Uneven chunk split (tc rows per chunk, summing to F=128). A small final
# chunk shrinks the post-DMA tail (last cast + matmuls before the softmax).
CHUNK_ROWS = None  # e.g. [40, 40, 40, 8]; None -> F // G_LOADS uniform
QBUFS = 3  # fp16 q tile pool depth
CASTSPLIT = 2  # sub-split of the LAST chunk's ScalarE cast (tail pipelining)
STORESPLIT = 1  # split final scale+store into halves (earlier store issue)
MULPROBE = False  # timing-only: skip the phase-2 multiply (WRONG outputs)


def build_nc(ns=NS, t=T, c=C, l=L, variant=None, repeat=1):
    variant = VARIANT if variant is None else variant
    key = (
        ns, t, c, l, variant, repeat, G_LOADS, DUAL_RING,
        tuple(CHUNK_ROWS) if CHUNK_ROWS else None, QBUFS, CASTSPLIT, STORESPLIT,
        MULPROBE, tuple(RINGS), V3_G, tuple(V3_RINGS), V3_TREE, V3_CASTSPLIT,
        V3_BCAST, V3_PROBE, V3_LOOKAHEAD, DMAX_KLOADS, V3_NOK, V3_KHW, V3_KMODE,
        V3_KHOIST, V3_STORE, V3_ENG,
    )
    if key in _NC_CACHE:
        return _NC_CACHE[key]

    f32 = mybir.dt.float32
    tc_sz = t * c
    F = tc_sz // P  # tc rows per partition
    assert tc_sz % P == 0
    # k/out per-partition regrouping: partition p holds flat tc in
    # [p*F, (p+1)*F); requires c % F == 0 or F % c == 0.
    nc = bacc.Bacc("TRN2", target_bir_lowering=False, debug=False)
    q_d = nc.dram_tensor("q", [ns, t, c, l], f32, kind="ExternalInput")
    k_d = nc.dram_tensor("k", [t, ns, c], f32, kind="ExternalInput")
    o_d = nc.dram_tensor("o", [t, ns, c], f32, kind="ExternalOutput")

    body = {
        "fp32": _body,
        "fp16": _body_fp16,
        "fp16hw": _body_fp16hw,
        "dmaonly": _body_dmaonly,
        "dmahw": _body_dmahw,
        "dmax": _body_dmax,
        "v3": _body_v3,
        "veng": _body_veng,
        "noop": _body_noop,
    }[variant]
    with tile.TileContext(nc) as tc_ctx:
        if repeat == 1:
            body(tc_ctx, q_d, k_d, o_d, ns, t, c, l)
        else:
            pre = pre_pools = None
            if variant == "v3" and V3_KHOIST:
                pre, pre_pools = _v3_k_prelude(tc_ctx, k_d, ns, t, c, l)
            elif variant == "veng":
                pre, pre_pools = _v3_eng_prelude(tc_ctx, q_d, k_d, ns, t, c, l)
            # PE body far exceeds one IRAM block; hint the back-edge prefetch.
            with tc_ctx.For_i(
                0, repeat, 1, hint_engines=(mybir.EngineType.PE,)
            ):
                if pre is not None and variant == "v3":
                    _body_v3(tc_ctx, q_d, k_d, o_d, ns, t, c, l, k_pre=pre)
                elif pre is not None:
                    _body_veng(tc_ctx, q_d, k_d, o_d, ns, t, c, l, pre=pre)
                else:
                    body(tc_ctx, q_d, k_d, o_d, ns, t, c, l)
            if pre_pools is not None:
                for p_ in reversed(pre_pools):
                    p_.release()
    nc.compile()
    _NC_CACHE[key] = nc
    return nc


def _body_fp16(tc_ctx, q_d, k_d, o_d, ns, t, c, l):
    """fp16 compute path: q/k cast to fp16 during the SWDGE load; fp16
    matmuls for scores; fp16 multiply + pairwise-tree reduce for phase 2.
    Softmax itself stays fp32 (PSUM accumulation is fp32)."""
    from contextlib import ExitStack

    nc = tc_ctx.nc
    f32 = mybir.dt.float32
    f16 = mybir.dt.float16
    tc_sz = t * c
    F = tc_sz // P

    stack = ExitStack()
    qpool = stack.enter_context(tc_ctx.tile_pool(name="qpool", bufs=3))
    kpool = stack.enter_context(tc_ctx.tile_pool(name="kpool", bufs=2))
    small = stack.enter_context(tc_ctx.tile_pool(name="small", bufs=4))
    opool = stack.enter_context(tc_ctx.tile_pool(name="opool", bufs=2))
    pscore = stack.enter_context(tc_ctx.tile_pool(name="pscore", bufs=2, space="PSUM"))

    G = 4  # q load chunks per sample (phase-1 overlaps the load)
    Fc = F // G

    for i in range(ns):
        # ---- loads (SWDGE casts f32 -> fp16 in the DMA), chunked ----
        k_t = kpool.tile([P, F], f16)
        nc.gpsimd.dma_start(
            out=k_t[:], in_=_flat_sample_kc(k_d.ap()[:, i, :], t, c, F)
        )

        q_src = q_d.ap()[i].rearrange("t c l -> (t c l)").rearrange(
            "(p x) -> p x", p=P
        )
        q_t = qpool.tile([P, F * l], f16)
        for g in range(G):
            nc.gpsimd.dma_start(
                out=q_t[:, g * Fc * l : (g + 1) * Fc * l],
                in_=q_src[:, g * Fc * l : (g + 1) * Fc * l],
            )

        # ---- phase 1: scores[l] = sum_tc q*k  (PE fp16, PSUM fp32) ----
        q3 = q_t[:].rearrange("p (f l) -> p f l", l=l)
        ps = pscore.tile([1, l], f32)
        for f in range(F):
            nc.tensor.matmul(
                ps[:],
                lhsT=k_t[:, f : f + 1],
                rhs=q3[:, f, :],
                start=(f == 0),
                stop=(f == F - 1),
            )

        # ---- softmax on [1, l] (fp32) ----
        negmax = small.tile([1, 1], f32)
        nc.vector.tensor_reduce(
            out=negmax[:], in_=ps[:], axis=mybir.AxisListType.X,
            op=mybir.AluOpType.max, negate=True,
        )
        exps16 = small.tile([1, l], f16)
        sumexp = small.tile([1, 1], f32)
        nc.scalar.activation(
            out=exps16[:], in_=ps[:], func=mybir.ActivationFunctionType.Exp,
            bias=negmax[:], scale=1.0, accum_out=sumexp[:],
        )
        # Deferred normalization: multiply by exp now, scale by 1/sum at the
        # end (keeps the reciprocal off the critical path).
        rsum = small.tile([1, 1], f32)
        nc.vector.reciprocal(out=rsum[:], in_=sumexp[:])
        rrep = small.tile([P, 1], f32)
        nc.gpsimd.partition_broadcast(rrep[:], rsum[:])

        # broadcast exp to all partitions
        srep = small.tile([P, l], f16)
        nc.gpsimd.partition_broadcast(srep[:], exps16[:])

        # ---- phase 2: q *= s, then pairwise tree-sum over l ----
        s_b = srep[:].unsqueeze(1).to_broadcast([P, F, l])
        nc.vector.tensor_tensor(out=q3, in0=q3, in1=s_b, op=mybir.AluOpType.mult)
        hh = l // 2
        while hh >= 2:
            nc.vector.tensor_tensor(
                out=q3[:, :, 0:hh],
                in0=q3[:, :, 0:hh],
                in1=q3[:, :, hh : 2 * hh],
                op=mybir.AluOpType.add,
            )
            hh //= 2
        ored = opool.tile([P, F], f32)
        nc.vector.tensor_tensor(
            out=ored[:],
            in0=q3[:, :, 0],
            in1=q3[:, :, 1],
            op=mybir.AluOpType.add,
        )
        # deferred softmax normalization
        nc.vector.tensor_scalar_mul(out=ored[:], in0=ored[:], scalar1=rrep[:])

        # ---- store ----
        nc.sync.dma_start(
            out=_flat_sample_kc(o_d.ap()[:, i, :], t, c, F), in_=ored[:]
        )

    stack.close()


def _flat_sample_kc(ap2d, t, c, F):
    """[t, c] AP -> AP iterating flat tc grouped as P partitions x F.

    Returned AP may be 3-dim; DMA matches flat element order, so it pairs
    with a [P, F] SBUF tile.
    """
    if c % F == 0:
        hh = c // F
        return ap2d.rearrange("t (hh f) -> t hh f", hh=hh)
    else:
        assert F % c == 0
        g = F // c  # whole t-rows per partition
        return ap2d.rearrange("(p g) c -> p (g c)", g=g)


def _body(tc_ctx, q_d, k_d, o_d, ns, t, c, l):
    from contextlib import ExitStack

    nc = tc_ctx.nc
    f32 = mybir.dt.float32
    tc_sz = t * c
    F = tc_sz // P

    stack = ExitStack()
    qpool = stack.enter_context(tc_ctx.tile_pool(name="qpool", bufs=3))
    kpool = stack.enter_context(tc_ctx.tile_pool(name="kpool", bufs=2))
    small = stack.enter_context(tc_ctx.tile_pool(name="small", bufs=4))
    opool = stack.enter_context(tc_ctx.tile_pool(name="opool", bufs=2))
    pscore = stack.enter_context(tc_ctx.tile_pool(name="pscore", bufs=2, space="PSUM"))

    for i in range(ns):
        # ---- loads ----
        q_t = qpool.tile([P, F * l], f32)
        q_src = q_d.ap()[i].rearrange("t c l -> (t c l)").rearrange(
            "(p x) -> p x", p=P
        )
        nc.sync.dma_start(out=q_t[:], in_=q_src)

        k_t = kpool.tile([P, F], f32)
        nc.sync.dma_start(out=k_t[:], in_=_flat_sample_kc(k_d.ap()[:, i, :], t, c, F))

        # ---- phase 1: scores[l] = sum_tc q*k  (PE, PSUM accumulation) ----
        q3 = q_t[:].rearrange("p (f l) -> p f l", l=l)
        ps = pscore.tile([1, l], f32)
        for f in range(F):
            nc.tensor.matmul(
                ps[:],
                lhsT=k_t[:, f : f + 1],
                rhs=q3[:, f, :],
                start=(f == 0),
                stop=(f == F - 1),
            )

        # ---- softmax on [1, l] ----
        negmax = small.tile([1, 1], f32)
        nc.vector.tensor_reduce(
            out=negmax[:], in_=ps[:], axis=mybir.AxisListType.X,
            op=mybir.AluOpType.max, negate=True,
        )
        exps = small.tile([1, l], f32)
        sumexp = small.tile([1, 1], f32)
        nc.scalar.activation(
            out=exps[:], in_=ps[:], func=mybir.ActivationFunctionType.Exp,
            bias=negmax[:], scale=1.0, accum_out=sumexp[:],
        )
        rsum = small.tile([1, 1], f32)
        nc.vector.reciprocal(out=rsum[:], in_=sumexp[:])
        srow = small.tile([1, l], f32)
        nc.vector.tensor_scalar_mul(out=srow[:], in0=exps[:], scalar1=rsum[:])

        # broadcast s to all partitions
        srep = small.tile([P, l], f32)
        nc.gpsimd.partition_broadcast(srep[:], srow[:])

        # ---- phase 2: q *= s (broadcast over f) ; reduce over l ----
        s_b = srep[:].unsqueeze(1).to_broadcast([P, F, l])
        nc.vector.tensor_tensor(
            out=q3, in0=q3, in1=s_b, op=mybir.AluOpType.mult
        )
        ored = opool.tile([P, F], f32)
        nc.vector.tensor_reduce(
            out=ored[:], in_=q3, axis=mybir.AxisListType.X,
            op=mybir.AluOpType.add,
        )

        # ---- store ----
        nc.sync.dma_start(
            out=_flat_sample_kc(o_d.ap()[:, i, :], t, c, F), in_=ored[:]
        )

    stack.close()


def _body_dmaonly(tc_ctx, q_d, k_d, o_d, ns, t, c, l):
    """Timing probe: SWDGE cast loads only, no compute."""
    from contextlib import ExitStack

    nc = tc_ctx.nc
    f32 = mybir.dt.float32
    f16 = mybir.dt.float16
    F = (t * c) // P
    stack = ExitStack()
    qpool = stack.enter_context(tc_ctx.tile_pool(name="qpool", bufs=3))
    kpool = stack.enter_context(tc_ctx.tile_pool(name="kpool", bufs=2))
    opool = stack.enter_context(tc_ctx.tile_pool(name="opool", bufs=2))
    G = 4
    Fc = F // G
    for i in range(ns):
        k_t = kpool.tile([P, F], f16)
        nc.gpsimd.dma_start(
            out=k_t[:], in_=_flat_sample_kc(k_d.ap()[:, i, :], t, c, F)
        )
        q_src = q_d.ap()[i].rearrange("t c l -> (t c l)").rearrange(
            "(p x) -> p x", p=P
        )
        q_t = qpool.tile([P, F * l], f16)
        for g in range(G):
            nc.gpsimd.dma_start(
                out=q_t[:, g * Fc * l : (g + 1) * Fc * l],
                in_=q_src[:, g * Fc * l : (g + 1) * Fc * l],
            )
        ored = opool.tile([P, F], f32)
        # touch the loaded tile so the store depends on the loads
        nc.vector.tensor_copy(ored[:], q_t[:].rearrange("p (f l) -> p f l", l=l)[:, :, 0])
        nc.sync.dma_start(
            out=_flat_sample_kc(o_d.ap()[:, i, :], t, c, F), in_=ored[:]
        )
    stack.close()


def _body_noop(tc_ctx, q_d, k_d, o_d, ns, t, c, l):
    """Calibration probe: near-empty body to measure For_i loop overhead."""
    from contextlib import ExitStack

    nc = tc_ctx.nc
    f32 = mybir.dt.float32
    F = (t * c) // P
    stack = ExitStack()
    opool = stack.enter_context(tc_ctx.tile_pool(name="opool", bufs=2))
    ored = opool.tile([P, F], f32)
    nc.vector.memset(ored[:], 0.0)
    nc.sync.dma_start(
        out=_flat_sample_kc(o_d.ap()[:, 0, :], t, c, F), in_=ored[:]
    )
    stack.close()


def _body_dmahw(tc_ctx, q_d, k_d, o_d, ns, t, c, l):
    """Timing probe: HWDGE f32 loads only, no cast/compute."""
    from contextlib import ExitStack

    nc = tc_ctx.nc
    f32 = mybir.dt.float32
    F = (t * c) // P
    stack = ExitStack()
    q32pool = stack.enter_context(tc_ctx.tile_pool(name="q32pool", bufs=3))
    opool = stack.enter_context(tc_ctx.tile_pool(name="opool", bufs=2))
    G = G_LOADS
    Fc = F // G
    for i in range(ns):
        q_src = q_d.ap()[i].rearrange("t c l -> (t c l)").rearrange(
            "(p x) -> p x", p=P
        )
        q32 = q32pool.tile([P, F * l], f32)
        for g in range(G):
            sl = slice(g * Fc * l, (g + 1) * Fc * l)
            eng = nc.scalar if (DUAL_RING and g % 2) else nc.sync
            eng.dma_start(out=q32[:, sl], in_=q_src[:, sl])
        ored = opool.tile([P, F], f32)
        nc.vector.tensor_copy(
            ored[:], q32[:].rearrange("p (f l) -> p f l", l=l)[:, :, 0]
        )
        nc.scalar.dma_start(
            out=_flat_sample_kc(o_d.ap()[:, i, :], t, c, F), in_=ored[:]
        )
    stack.close()


RINGS = ("sp", "act")  # per-chunk ring rotation for the dmax probe

# ---- v3 knobs ----
V3_G = 6  # q chunks per sample
V3_RINGS = ("sp", "act", "gpcast")  # chunk ring rotation
V3_TREE = True  # pairwise tree reduce (False: single tensor_reduce)
V3_CASTSPLIT = 1  # sub-splits of each ScalarE cast chunk
V3_LOOKAHEAD = 2  # how many samples of q DMA issue to run ahead
V3_BCAST = "gp"  # softmax broadcast path: 'pe' (matmul+ACT copy) | 'gp' (partition_broadcast)
DMAX_KLOADS = False  # add k SWDGE loads to the dmax probe
V3_KHW = False  # load k via HWDGE f32 + ScalarE cast (not SWDGE cast-DMA)
V3_KMODE = "pe"  # 'swdge' | 'khw' | 'shuffle' | 'pe' (contig k_all + PE permutation matmuls)
V3_KHOIST = True  # hoist the (iteration-invariant) k pipeline out of the repeat loop
V3_STORE = "act_end"  # 'sp_inline' | 'act_end' (stores deferred to end of body on ACT ring)
V3_ENG = "all"  # veng probe: 'act' | 'pe' | 'p2' | 'dve' | 'gp' | 'all'
V3_NOK = False  # skip k loads (dmas probe bisection)
V3_PROBE = "full"  # timing probes: 'full' | 'dmas' | 'nosm' | 'nop2' (non-full = WRONG outputs)


def _v3_chunks(F):
    rows = [F // V3_G + (1 if g < F % V3_G else 0) for g in range(V3_G)]
    chunks = []
    r0 = 0
    hw_off = 0
    for g in range(V3_G):
        ring = V3_RINGS[g % len(V3_RINGS)]
        r1 = r0 + rows[g]
        if ring in ("sp", "act"):
            chunks.append((ring, r0, r1, hw_off))
            hw_off += r1 - r0
        else:
            chunks.append((ring, r0, r1, None))
        r0 = r1
    return chunks, hw_off




def _v3_eng_prelude(tc_ctx, q_d, k_d, ns, t, c, l):
    """Load everything once so the repeat loop can exercise single engines."""
    nc = tc_ctx.nc
    f32 = mybir.dt.float32
    f16 = mybir.dt.float16
    F = (t * c) // P
    pool = tc_ctx.alloc_tile_pool(name="vepool", bufs=1)
    pspool = tc_ctx.alloc_tile_pool(name="vepsum", bufs=4, space="PSUM")
    pre = {"q32": [], "q16": [], "k": [], "ps": [], "srep": [], "o": []}
    k_ts, kpools = _v3_k_prelude(tc_ctx, k_d, ns, t, c, l)
    pre["k"] = k_ts
    for i in range(ns):
        q_src = q_d.ap()[i].rearrange("t c l -> (t c l)").rearrange(
            "(p x) -> p x", p=P
        )
        q32 = pool.tile([P, F * l], f32, tag=f"q32_{i}", name=f"vq32_{i}")
        nc.sync.dma_start(out=q32[:], in_=q_src)
        q16 = pool.tile([P, F * l], f16, tag=f"q16_{i}", name=f"vq16_{i}")
        nc.scalar.copy(out=q16[:], in_=q32[:])
        ps = pspool.tile([1, l], f32, tag="ps", name=f"vps{i}", bufs=4)
        nc.vector.memset(ps[:], 0.25)
        srep = pool.tile([P, l], f16, tag=f"sr{i}", name=f"vsr{i}")
        nc.vector.memset(srep[:], 0.01)
        ored = pool.tile([P, F], f32, tag=f"o{i}", name=f"vo{i}")
        nc.vector.memset(ored[:], 0.0)
        pre["q32"].append(q32)
        pre["q16"].append(q16)
        pre["ps"].append(ps)
        pre["srep"].append(srep)
        pre["o"].append(ored)
    return pre, (pool, pspool) + kpools


def _body_veng(tc_ctx, q_d, k_d, o_d, ns, t, c, l, pre=None):
    """Engine-isolated compute probe (V3_ENG selects the work)."""
    nc = tc_ctx.nc
    f32 = mybir.dt.float32
    f16 = mybir.dt.float16
    F = (t * c) // P
    eng = V3_ENG
    small = tc_ctx.alloc_tile_pool(name="vsmall", bufs=4)

    for i in range(ns):
        q32, q16 = pre["q32"][i], pre["q16"][i]
        k_t, ps, srep, ored = pre["k"][i], pre["ps"][i], pre["srep"][i], pre["o"][i]
        q3 = q16[:].rearrange("p (f l) -> p f l", l=l)

        if eng in ("act", "all"):
            half = (F // 2) * l
            nc.scalar.copy(out=q16[:, :half], in_=q32[:, :half])
            nc.scalar.copy(out=q16[:, half:], in_=q32[:, half:])

        if eng in ("pe", "all"):
            for f in range(F):
                nc.tensor.matmul(
                    ps[:], lhsT=k_t[:, f : f + 1], rhs=q3[:, f, :],
                    start=(f == 0), stop=(f == F - 1),
                )

        if eng in ("dve", "all"):
            negmax = small.tile([1, 1], f32, tag="negmax")
            nc.vector.tensor_reduce(
                out=negmax[:], in_=ps[:], axis=mybir.AxisListType.X,
                op=mybir.AluOpType.max, negate=True,
            )
            exps16 = small.tile([1, l], f16, tag="exps")
            sumexp = small.tile([1, 1], f32, tag="sumexp")
            if eng == "all":
                nc.scalar.activation(
                    out=exps16[:], in_=ps[:],
                    func=mybir.ActivationFunctionType.Exp,
                    bias=negmax[:], scale=1.0, accum_out=sumexp[:],
                )
            else:
                nc.vector.memset(exps16[:], 0.5)
                nc.vector.memset(sumexp[:], 32.0)
            rsum = small.tile([1, 1], f32, tag="rsum")
            nc.vector.reciprocal(out=rsum[:], in_=sumexp[:])
            snorm = small.tile([1, l], f16, tag="snorm")
            nc.vector.tensor_scalar_mul(out=snorm[:], in0=exps16[:], scalar1=rsum[:])
            if eng == "all" or V3_BCAST == "gp":
                nc.gpsimd.partition_broadcast(srep[:], snorm[:])

        if eng == "gp":
            snorm = small.tile([1, l], f16, tag="snorm")
            nc.vector.memset(snorm[:], 0.5)
            nc.gpsimd.partition_broadcast(srep[:], snorm[:])

        if eng in ("p2", "dve", "all"):
            s_b = srep[:].unsqueeze(1).to_broadcast([P, F, l])
            nc.vector.tensor_tensor(out=q3, in0=q3, in1=s_b, op=mybir.AluOpType.mult)
            if V3_TREE:
                hh = l // 2
                while hh >= 2:
                    nc.vector.tensor_tensor(
                        out=q3[:, :, 0:hh], in0=q3[:, :, 0:hh],
                        in1=q3[:, :, hh : 2 * hh], op=mybir.AluOpType.add,
                    )
                    hh //= 2
                nc.vector.tensor_tensor(
                    out=ored[:], in0=q3[:, :, 0], in1=q3[:, :, 1],
                    op=mybir.AluOpType.add,
                )
            else:
                nc.vector.tensor_reduce(
                    out=ored[:], in_=q3, axis=mybir.AxisListType.X,
                    op=mybir.AluOpType.add,
                )
    small.release()


def _v3_k_prelude(tc_ctx, k_d, ns, t, c, l):
    """Iteration-invariant k pipeline, traced once before the repeat loop:
    contiguous kall load + PE permutation matmuls + ScalarE casts."""
    nc = tc_ctx.nc
    f32 = mybir.dt.float32
    f16 = mybir.dt.float16
    F = (t * c) // P
    kpool = tc_ctx.alloc_tile_pool(name="kprepool", bufs=1)
    kppool = tc_ctx.alloc_tile_pool(name="kprepsum", bufs=2, space="PSUM")

    kall = kpool.tile([t, ns * c], f32, tag="kall", name="kall")
    nc.scalar.dma_start(out=kall[:], in_=k_d.ap().rearrange("t n c -> t (n c)"))

    ei = kpool.tile([t, P], mybir.dt.int32, tag="ei", name="ei")
    nc.gpsimd.iota(out=ei[:], pattern=[[1, P]], base=0, channel_multiplier=-2)
    eperm = []
    for pb in range(2):
        e = kpool.tile([t, P], f32, tag=f"e{pb}", name=f"e{pb}")
        nc.vector.tensor_scalar(
            out=e[:], in0=ei[:], scalar1=pb, scalar2=None,
            op0=mybir.AluOpType.is_equal,
        )
        eperm.append(e)

    k_ts = []
    for i in range(ns):
        k_t = kpool.tile([P, F], f16, tag=f"k{i}", name=f"k{i}")
        kp = kppool.tile([P, F], f32, tag="kp", name=f"kp{i}")
        nc.tensor.matmul(
            kp[:], lhsT=eperm[0][:], rhs=kall[:, i * c : i * c + F],
            start=True, stop=False,
        )
        nc.tensor.matmul(
            kp[:], lhsT=eperm[1][:], rhs=kall[:, i * c + F : (i + 1) * c],
            start=False, stop=True,
        )
        nc.scalar.copy(out=k_t[:], in_=kp[:])
        k_ts.append(k_t)
    return k_ts, (kpool, kppool)


def _body_v3(tc_ctx, q_d, k_d, o_d, ns, t, c, l, k_pre=None):
    """Three-path q loads (SP/ACT HWDGE f32 + SWDGE fp16-cast), emission
    software-pipelined: q DMAs run V3_LOOKAHEAD samples ahead and ScalarE
    casts one sample ahead of each sample's softmax, so no engine queue
    ping-pongs between DMA issue / cast / exp. Softmax is normalized up
    front; broadcast via gpsimd partition_broadcast ('gp') or a PE rank-1
    matmul + ACT copy ('pe')."""
    from contextlib import ExitStack

    nc = tc_ctx.nc
    f32 = mybir.dt.float32
    f16 = mybir.dt.float16
    F = (t * c) // P
    chunks, HW = _v3_chunks(F)

    stack = ExitStack()
    q32pool = stack.enter_context(tc_ctx.tile_pool(name="q32pool", bufs=1))
    q16pool = stack.enter_context(tc_ctx.tile_pool(name="q16pool", bufs=1))
    kpool = stack.enter_context(tc_ctx.tile_pool(name="kpool", bufs=2))
    small = stack.enter_context(tc_ctx.tile_pool(name="small", bufs=4))
    srpool = stack.enter_context(tc_ctx.tile_pool(name="srpool", bufs=2))
    opool = stack.enter_context(tc_ctx.tile_pool(name="opool", bufs=2))
    pscore = stack.enter_context(
        tc_ctx.tile_pool(name="pscore", bufs=2, space="PSUM")
    )
    psrep = stack.enter_context(
        tc_ctx.tile_pool(name="psrep", bufs=2, space="PSUM")
    )
    kppool = stack.enter_context(
        tc_ctx.tile_pool(name="kppool", bufs=2, space="PSUM")
    )

    q32s = [None] * ns
    q16s = [None] * ns
    k_ts = [None] * ns
    pending_stores = []

    k32s = [None] * ns
    kall = [None]

    def emit_kall():
        # whole per-core k in one contiguous HBM load: partition a = t row a,
        # 4 KB lines (64 descriptors)
        ka = kpool.tile([t, ns * c], f32, tag="kall", name="kall")
        nc.scalar.dma_start(
            out=ka[:], in_=k_d.ap().rearrange("t n c -> t (n c)")
        )
        kall[0] = ka

    eperm = [None, None]

    def emit_eperm():
        # E_pb[a, m] = 1.0 iff m == 2a + pb  (partition-pair shuffle operands)
        ei = kpool.tile([t, P], mybir.dt.int32, tag="ei", name="ei")
        nc.gpsimd.iota(
            out=ei[:], pattern=[[1, P]], base=0, channel_multiplier=-2
        )
        for pb in range(2):
            e = kpool.tile([t, P], f32, tag=f"e{pb}", name=f"e{pb}")
            nc.vector.tensor_scalar(
                out=e[:], in0=ei[:], scalar1=pb, scalar2=None,
                op0=mybir.AluOpType.is_equal,
            )
            eperm[pb] = e

    def emit_kload(i):
        k_t = kpool.tile([P, F], f16, tag=f"k{i}", name=f"k{i}")
        if V3_KMODE == "pe":
            kp = kppool.tile([P, F], f32, tag="kp", name=f"kp{i}")
            nc.tensor.matmul(
                kp[:], lhsT=eperm[0][:], rhs=kall[0][:, i * c : i * c + F],
                start=True, stop=False,
            )
            nc.tensor.matmul(
                kp[:], lhsT=eperm[1][:], rhs=kall[0][:, i * c + F : (i + 1) * c],
                start=False, stop=True,
            )
            nc.scalar.copy(out=k_t[:], in_=kp[:])
        elif V3_KMODE == "shuffle":
            # SBUF->SBUF SWDGE cast + partition shuffle out of kall:
            # k_t[2a+pb, f] = kall[a, i*c + pb*F + f]
            nc.gpsimd.dma_start(
                out=k_t[:].rearrange("(a pb) f -> a pb f", pb=2),
                in_=kall[0][:, i * c : (i + 1) * c].rearrange(
                    "a (pb f) -> a pb f", f=F
                ),
            )
        elif V3_KMODE == "khw" or V3_KHW:
            k32 = kpool.tile([P, F], f32, tag=f"k32_{i}", name=f"k32_{i}")
            nc.sync.dma_start(
                out=k32[:], in_=_flat_sample_kc(k_d.ap()[:, i, :], t, c, F)
            )
            k32s[i] = k32
        else:
            nc.gpsimd.dma_start(
                out=k_t[:], in_=_flat_sample_kc(k_d.ap()[:, i, :], t, c, F)
            )
        k_ts[i] = k_t

    def emit_qdma(i):
        q_src = q_d.ap()[i].rearrange("t c l -> (t c l)").rearrange(
            "(p x) -> p x", p=P
        )
        q32 = q32pool.tile([P, HW * l], f32, tag=f"q32_{i}", name=f"q32_{i}")
        q16 = q16pool.tile([P, F * l], f16, tag=f"q16_{i}", name=f"q16_{i}")
        for ring, r0, r1, off in chunks:
            src = q_src[:, r0 * l : r1 * l]
            if ring == "sp":
                nc.sync.dma_start(out=q32[:, off * l : (off + r1 - r0) * l], in_=src)
            elif ring == "act":
                nc.scalar.dma_start(out=q32[:, off * l : (off + r1 - r0) * l], in_=src)
            else:
                nc.gpsimd.dma_start(out=q16[:, r0 * l : r1 * l], in_=src)
        q32s[i] = q32
        q16s[i] = q16

    def emit_casts(i):
        q16, q32 = q16s[i], q32s[i]
        for ring, r0, r1, off in chunks:
            if off is None:
                continue
            nr = r1 - r0
            sub = nr // V3_CASTSPLIT
            subs = [sub] * V3_CASTSPLIT
            subs[-1] += nr - sub * V3_CASTSPLIT
            s0 = 0
            for srws in subs:
                nc.scalar.copy(
                    out=q16[:, (r0 + s0) * l : (r0 + s0 + srws) * l],
                    in_=q32[:, (off + s0) * l : (off + s0 + srws) * l],
                )
                s0 += srws

    # prologue: all k loads, then the pipelined head
    if k_pre is not None:
        for i in range(ns):
            k_ts[i] = k_pre[i]
    elif not V3_NOK:
        if V3_KMODE in ("shuffle", "pe"):
            emit_kall()
        if V3_KMODE == "pe":
            emit_eperm()
        for i in range(ns):
            emit_kload(i)
    look = min(V3_LOOKAHEAD, ns)
    for i in range(look):
        emit_qdma(i)
    if V3_PROBE == "dmas":
        for i in range(look, ns):
            emit_qdma(i)
        for i in range(ns):
            q16, q32 = q16s[i], q32s[i]
            ored = opool.tile([P, F], f32, tag="ored", name=f"ored{i}")
            nc.vector.tensor_copy(
                ored[:], q16[:].rearrange("p (f l) -> p f l", l=l)[:, :, 0]
            )
            nc.vector.tensor_tensor(
                out=ored[:, :HW], in0=ored[:, :HW],
                in1=q32[:].rearrange("p (f l) -> p f l", l=l)[:, :, 0],
                op=mybir.AluOpType.add,
            )
            nc.sync.dma_start(
                out=_flat_sample_kc(o_d.ap()[:, i, :], t, c, F), in_=ored[:]
            )
        stack.close()
        return

    if V3_BCAST == "pe":
        ones = small.tile([1, P], f16, tag="ones")
        nc.vector.memset(ones[:], 1.0)
    if (V3_KMODE == "khw" or V3_KHW) and V3_KMODE != "shuffle" and not V3_NOK:
        for i in range(ns):
            nc.scalar.copy(out=k_ts[i][:], in_=k32s[i][:])
    emit_casts(0)

    for i in range(ns):
        if i + look < ns:
            emit_qdma(i + look)
        if i + 1 < ns:
            emit_casts(i + 1)

        q16, q32, k_t = q16s[i], q32s[i], k_ts[i]
        # phase 1
        q3 = q16[:].rearrange("p (f l) -> p f l", l=l)
        ps = pscore.tile([1, l], f32, tag="ps", name=f"ps{i}")
        for f in range(F):
            nc.tensor.matmul(
                ps[:],
                lhsT=k_t[:, f : f + 1],
                rhs=q3[:, f, :],
                start=(f == 0),
                stop=(f == F - 1),
            )

        if V3_PROBE == "nosm":
            srep = srpool.tile([P, l], f16, tag="srep", name=f"srep{i}")
            nc.vector.memset(srep[:], 0.01)
        else:
            # softmax (normalized up front)
            negmax = small.tile([1, 1], f32, tag="negmax")
            nc.vector.tensor_reduce(
                out=negmax[:], in_=ps[:], axis=mybir.AxisListType.X,
                op=mybir.AluOpType.max, negate=True,
            )
            exps16 = small.tile([1, l], f16, tag="exps")
            sumexp = small.tile([1, 1], f32, tag="sumexp")
            nc.scalar.activation(
                out=exps16[:], in_=ps[:], func=mybir.ActivationFunctionType.Exp,
                bias=negmax[:], scale=1.0, accum_out=sumexp[:],
            )
            rsum = small.tile([1, 1], f32, tag="rsum")
            nc.vector.reciprocal(out=rsum[:], in_=sumexp[:])
            snorm = small.tile([1, l], f16, tag="snorm")
            nc.vector.tensor_scalar_mul(out=snorm[:], in0=exps16[:], scalar1=rsum[:])
            srep = srpool.tile([P, l], f16, tag="srep", name=f"srep{i}")
            if V3_BCAST == "pe":
                psr = psrep.tile([P, l], f32, tag="psr", name=f"psr{i}")
                nc.tensor.matmul(
                    psr[:], lhsT=ones[:], rhs=snorm[:], start=True, stop=True
                )
                nc.scalar.copy(out=srep[:], in_=psr[:])
            else:
                nc.gpsimd.partition_broadcast(srep[:], snorm[:])

        # phase 2
        ored = opool.tile([P, F], f32, tag="ored", name=f"ored{i}")
        if V3_PROBE == "nop2":
            nc.vector.tensor_copy(ored[:], q3[:, :, 0])
            nc.vector.tensor_scalar_mul(
                out=ored[:], in0=ored[:], scalar1=srep[:, 0:1]
            )
        else:
            s_b = srep[:].unsqueeze(1).to_broadcast([P, F, l])
            nc.vector.tensor_tensor(out=q3, in0=q3, in1=s_b, op=mybir.AluOpType.mult)
            if V3_TREE:
                hh = l // 2
                while hh >= 2:
                    nc.vector.tensor_tensor(
                        out=q3[:, :, 0:hh],
                        in0=q3[:, :, 0:hh],
                        in1=q3[:, :, hh : 2 * hh],
                        op=mybir.AluOpType.add,
                    )
                    hh //= 2
                nc.vector.tensor_tensor(
                    out=ored[:], in0=q3[:, :, 0], in1=q3[:, :, 1],
                    op=mybir.AluOpType.add,
                )
            else:
                nc.vector.tensor_reduce(
                    out=ored[:], in_=q3, axis=mybir.AxisListType.X,
                    op=mybir.AluOpType.add,
                )

        # store
        out_ap = _flat_sample_kc(o_d.ap()[:, i, :], t, c, F)
        if V3_STORE == "act_end":
            pending_stores.append((out_ap, ored[:]))
        else:
            nc.sync.dma_start(out=out_ap, in_=ored[:])

    for ap_o, t_o in pending_stores:
        nc.scalar.dma_start(out=ap_o, in_=t_o)

    stack.close()


def _body_dmax(tc_ctx, q_d, k_d, o_d, ns, t, c, l):
    """Timing probe: q loads spread across rings per RINGS rotation.
    'sp'/'act' = HWDGE f32; 'gp' = SWDGE f32; 'gpcast' = SWDGE f32->fp16."""
    from contextlib import ExitStack

    nc = tc_ctx.nc
    f32 = mybir.dt.float32
    f16 = mybir.dt.float16
    F = (t * c) // P
    stack = ExitStack()
    q32pool = stack.enter_context(tc_ctx.tile_pool(name="q32pool", bufs=2))
    q16pool = stack.enter_context(tc_ctx.tile_pool(name="q16pool", bufs=2))
    opool = stack.enter_context(tc_ctx.tile_pool(name="opool", bufs=2))
    kpool = stack.enter_context(tc_ctx.tile_pool(name="kpool", bufs=2))
    if DMAX_KLOADS:
        for i in range(ns):
            k_t = kpool.tile([P, F], f16, tag=f"k{i}", name=f"k{i}")
            nc.gpsimd.dma_start(
                out=k_t[:], in_=_flat_sample_kc(k_d.ap()[:, i, :], t, c, F)
            )
    G = G_LOADS
    rows = [F // G + (1 if g < F % G else 0) for g in range(G)]
    bounds = [0]
    for r in rows:
        bounds.append(bounds[-1] + r)
    for i in range(ns):
        q_src = q_d.ap()[i].rearrange("t c l -> (t c l)").rearrange(
            "(p x) -> p x", p=P
        )
        q32 = q32pool.tile([P, F * l], f32)
        q16 = q16pool.tile([P, F * l], f16)
        touch = []
        for g in range(G):
            sl = slice(bounds[g] * l, bounds[g + 1] * l)
            ring = RINGS[g % len(RINGS)]
            if ring == "sp":
                nc.sync.dma_start(out=q32[:, sl], in_=q_src[:, sl])
                touch.append(q32)
            elif ring == "act":
                nc.scalar.dma_start(out=q32[:, sl], in_=q_src[:, sl])
                touch.append(q32)
            elif ring == "gp":
                nc.gpsimd.dma_start(out=q32[:, sl], in_=q_src[:, sl])
                touch.append(q32)
            elif ring == "gpcast":
                nc.gpsimd.dma_start(out=q16[:, sl], in_=q_src[:, sl])
                touch.append(q16)
        ored = opool.tile([P, F], f32)
        srcs = {id(x): x for x in touch}
        for j, x in enumerate(srcs.values()):
            if j == 0:
                nc.vector.tensor_copy(
                    ored[:], x[:].rearrange("p (f l) -> p f l", l=l)[:, :, 0]
                )
            else:
                nc.vector.tensor_tensor(
                    out=ored[:], in0=ored[:],
                    in1=x[:].rearrange("p (f l) -> p f l", l=l)[:, :, 0],
                    op=mybir.AluOpType.add,
                )
        nc.sync.dma_start(
            out=_flat_sample_kc(o_d.ap()[:, i, :], t, c, F), in_=ored[:]
        )
    stack.close()


def _body_fp16hw(tc_ctx, q_d, k_d, o_d, ns, t, c, l):
    """Like _body_fp16 but loads q as f32 via HWDGE (full DMA rate) and casts
    f32 -> fp16 on the (otherwise idle) ScalarE."""
    from contextlib import ExitStack

    nc = tc_ctx.nc
    f32 = mybir.dt.float32
    f16 = mybir.dt.float16
    F = (t * c) // P

    stack = ExitStack()
    q32pool = stack.enter_context(tc_ctx.tile_pool(name="q32pool", bufs=3))
    qpool = stack.enter_context(tc_ctx.tile_pool(name="qpool", bufs=QBUFS))
    kpool = stack.enter_context(tc_ctx.tile_pool(name="kpool", bufs=2))
    small = stack.enter_context(tc_ctx.tile_pool(name="small", bufs=4))
    opool = stack.enter_context(tc_ctx.tile_pool(name="opool", bufs=4))
    pscore = stack.enter_context(tc_ctx.tile_pool(name="pscore", bufs=2, space="PSUM"))

    rows = CHUNK_ROWS if CHUNK_ROWS else [F // G_LOADS] * G_LOADS
    assert sum(rows) == F
    bounds = [0]
    for r in rows:
        bounds.append(bounds[-1] + r)

    # All k loads upfront (SWDGE, tiny) so nothing later blocks them.
    k_ts = []
    for i in range(ns):
        k_t = kpool.tile([P, F], f16, tag=f"k{i}")
        nc.gpsimd.dma_start(
            out=k_t[:], in_=_flat_sample_kc(k_d.ap()[:, i, :], t, c, F)
        )
        k_ts.append(k_t)

    # Stores are emitted two samples late: a store's semaphore wait (on the
    # phase-2 result) must never block later q-load issues on the SP ring.
    pending_stores = []

    def flush_store():
        ap_out, tile_in = pending_stores.pop(0)
        nc.sync.dma_start(out=ap_out, in_=tile_in)

    for i in range(ns):
        k_t = k_ts[i]
        q_src = q_d.ap()[i].rearrange("t c l -> (t c l)").rearrange(
            "(p x) -> p x", p=P
        )
        q32 = q32pool.tile([P, F * l], f32)
        q_t = qpool.tile([P, F * l], f16)
        for g in range(len(rows)):
            sl = slice(bounds[g] * l, bounds[g + 1] * l)
            eng = nc.scalar if (DUAL_RING and g % 2) else nc.sync
            eng.dma_start(out=q32[:, sl], in_=q_src[:, sl])
            if g == len(rows) - 1 and CASTSPLIT > 1:
                # tail chunk: sub-split the cast so its matmuls pipeline
                # behind sub-casts instead of one long cast
                sub = rows[g] // CASTSPLIT
                for s_i in range(CASTSPLIT):
                    ss = slice(
                        (bounds[g] + s_i * sub) * l,
                        (bounds[g] + (s_i + 1) * sub) * l,
                    )
                    nc.scalar.copy(out=q_t[:, ss], in_=q32[:, ss])
            else:
                nc.scalar.copy(out=q_t[:, sl], in_=q32[:, sl])

        # ---- phase 1 ----
        q3 = q_t[:].rearrange("p (f l) -> p f l", l=l)
        ps = pscore.tile([1, l], f32)
        for f in range(F):
            nc.tensor.matmul(
                ps[:],
                lhsT=k_t[:, f : f + 1],
                rhs=q3[:, f, :],
                start=(f == 0),
                stop=(f == F - 1),
            )

        # ---- softmax (deferred normalization) ----
        negmax = small.tile([1, 1], f32)
        nc.vector.tensor_reduce(
            out=negmax[:], in_=ps[:], axis=mybir.AxisListType.X,
            op=mybir.AluOpType.max, negate=True,
        )
        exps16 = small.tile([1, l], f16)
        sumexp = small.tile([1, 1], f32)
        nc.scalar.activation(
            out=exps16[:], in_=ps[:], func=mybir.ActivationFunctionType.Exp,
            bias=negmax[:], scale=1.0, accum_out=sumexp[:],
        )
        # srep broadcast first (gates the phase-2 multiply); rrep is only
        # needed at the final scale, so it goes second on the POOL stream.
        srep = small.tile([P, l], f16)
        nc.gpsimd.partition_broadcast(srep[:], exps16[:])
        rsum = small.tile([1, 1], f32)
        nc.vector.reciprocal(out=rsum[:], in_=sumexp[:])
        rrep = small.tile([P, 1], f32)
        nc.gpsimd.partition_broadcast(rrep[:], rsum[:])

        # ---- phase 2 ----
        ored = opool.tile([P, F], f32)
        if MULPROBE == 2:
            # timing probe: no phase-2 at all (WRONG outputs)
            nc.vector.tensor_copy(ored[:], q3[:, :, 0])
        else:
            if not MULPROBE:
                s_b = srep[:].unsqueeze(1).to_broadcast([P, F, l])
                nc.vector.tensor_tensor(
                    out=q3, in0=q3, in1=s_b, op=mybir.AluOpType.mult
                )
            hh = l // 2
            while hh >= 2:
                nc.vector.tensor_tensor(
                    out=q3[:, :, 0:hh],
                    in0=q3[:, :, 0:hh],
                    in1=q3[:, :, hh : 2 * hh],
                    op=mybir.AluOpType.add,
                )
                hh //= 2
            nc.vector.tensor_tensor(
                out=ored[:], in0=q3[:, :, 0], in1=q3[:, :, 1],
                op=mybir.AluOpType.add,
            )
        out_ap = _flat_sample_kc(o_d.ap()[:, i, :], t, c, F)
        if STORESPLIT > 1:
            half = F // 2
            nc.vector.tensor_scalar_mul(
                out=ored[:, :half], in0=ored[:, :half], scalar1=rrep[:]
            )
            pending_stores.append((out_ap[:, :, :half], ored[:, :half]))
            nc.vector.tensor_scalar_mul(
                out=ored[:, half:], in0=ored[:, half:], scalar1=rrep[:]
            )
            pending_stores.append((out_ap[:, :, half:], ored[:, half:]))
        else:
            nc.vector.tensor_scalar_mul(out=ored[:], in0=ored[:], scalar1=rrep[:])
            pending_stores.append((out_ap, ored[:]))
        while len(pending_stores) > 2 * STORESPLIT:
            flush_store()

    while pending_stores:
        flush_store()

    stack.close()


def run(query, key, repeat=1, variant=None, **spmd_kwargs):
    query = np.ascontiguousarray(np.asarray(query, dtype=np.float32))
    key = np.asarray(key, dtype=np.float32)
    n, t, c, l = query.shape
    ncores = NCORES
    ns = n // ncores
    nc = build_nc(ns, t, c, l, variant=variant, repeat=repeat)

    in_maps = []
    for i in range(ncores):
        in_maps.append(
            {
                "q": np.ascontiguousarray(query[i * ns : (i + 1) * ns]),
                "k": np.ascontiguousarray(key[:, i * ns : (i + 1) * ns, :]),
            }
        )
    res = bass_utils.run_bass_kernel_spmd(
        nc, in_maps, core_ids=list(range(ncores)), **spmd_kwargs
    )
    out = np.empty((t, n, c), dtype=np.float32)
    for i in range(ncores):
        out[:, i * ns : (i + 1) * ns, :] = res.results[i]["o"]
    return out, res


def kernel(**inputs):
    out, _ = run(inputs["query"], inputs["key"])
    return out



# revision 44
# speedup vs baseline: 1.0279x; 1.0279x over previous
"""Trainium2 Bass kernel for nn_Aggregation_Attn.

Computation (per sample i):
    scores[l] = sum_{t,c} q[i,t,c,l] * k[t,i,c]        # contraction over tc=t*c
    s         = softmax(scores)                         # over l
    out[t,c]  = sum_l q[i,t,c,l] * s[l]                 # contraction over l

Shapes: q [32, 64, 256, 64] f32, k [64, 32, 256] f32, out [64, 32, 256] f32.
Data-parallel over n across 8 cores (4 samples/core).

Per-core layout: q sample flattened to [tc=16384, l=64] (contiguous in HBM),
loaded into SBUF as [128 partitions, 8192] (partition p holds tc rows
[p*128, (p+1)*128), each partition row is one contiguous 32 KB HBM chunk).

Phase 1 on TensorE: for each tc0 in 0..127, matmul with stationary k column
k_sb[:, tc0] ([128,1]) and moving q slice [128, 64] -> accumulate psum [1, 64].
Softmax on tiny [1, 64] (DVE reduce_max -> ACT exp with accum -> DVE recip).
Phase 2 on VectorE: q *= s (broadcast over tc), then reduce over l.
"""

import numpy as np

import concourse.bacc as bacc
import concourse.bass as bass
import concourse.mybir as mybir
import concourse.tile as tile
from concourse import bass_utils

N, T, C, L = 32, 64, 256, 64
NCORES = 8
NS = N // NCORES  # samples per core
P = 128

_NC_CACHE = {}

VARIANT = "v6"
G_LOADS = 4  # q load chunks per sample (HW-measured optimum)
DUAL_RING = False  # alternate q-chunk DMAs between SP and ACT HWDGE rings
# Uneven chunk split (tc rows per chunk, summing to F=128). A small final
# chunk shrinks the post-DMA tail (last cast + matmuls before the softmax).
CHUNK_ROWS = None  # e.g. [40, 40, 40, 8]; None -> F // G_LOADS uniform
QBUFS = 3  # fp16 q tile pool depth
CASTSPLIT = 2  # sub-split of the LAST chunk's ScalarE cast (tail pipelining)
STORESPLIT = 1  # split final scale+store into halves (earlier store issue)
MULPROBE = False  # timing-only: skip the phase-2 multiply (WRONG outputs)


def build_nc(ns=NS, t=T, c=C, l=L, variant=None, repeat=1):
    variant = VARIANT if variant is None else variant
    key = (
        ns, t, c, l, variant, repeat, G_LOADS, DUAL_RING,
        tuple(CHUNK_ROWS) if CHUNK_ROWS else None, QBUFS, CASTSPLIT, STORESPLIT,
        MULPROBE, tuple(RINGS), V3_G, tuple(V3_RINGS), V3_TREE, V3_CASTSPLIT,
        V3_BCAST, V3_PROBE, V3_LOOKAHEAD, DMAX_KLOADS, V3_NOK, V3_KHW, V3_KMODE,
        V3_KHOIST, V3_STORE, V3_ENG,
        V6_G, V6_Q32BUFS, V6_Q16BUFS, V6_CASTSPLIT_LAST, V6_POOL_CHUNKS,
        V6_POOL_F, V6_BCAST, V6_STORE, V6_QAHEAD, V6_SCHED, V6_SMUL,
        V6_HIPRI_SM, V6_PROBE, V6_NORM, V6_PAIR,
        tuple(V6_ROWS) if V6_ROWS else None, V6_DIV, V6_PSBUFS,
    )
    if key in _NC_CACHE:
        return _NC_CACHE[key]

    f32 = mybir.dt.float32
    tc_sz = t * c
    F = tc_sz // P  # tc rows per partition
    assert tc_sz % P == 0
    # k/out per-partition regrouping: partition p holds flat tc in
    # [p*F, (p+1)*F); requires c % F == 0 or F % c == 0.
    nc = bacc.Bacc("TRN2", target_bir_lowering=False, debug=False)
    q_d = nc.dram_tensor("q", [ns, t, c, l], f32, kind="ExternalInput")
    k_d = nc.dram_tensor("k", [t, ns, c], f32, kind="ExternalInput")
    o_d = nc.dram_tensor("o", [t, ns, c], f32, kind="ExternalOutput")

    body = {
        "fp32": _body,
        "fp16": _body_fp16,
        "fp16hw": _body_fp16hw,
        "dmaonly": _body_dmaonly,
        "dmahw": _body_dmahw,
        "dmax": _body_dmax,
        "v3": _body_v3,
        "v6": _body_v6,
        "pair": _body_pair,
        "veng": _body_veng,
        "noop": _body_noop,
    }[variant]
    with tile.TileContext(nc) as tc_ctx:
        if repeat == 1:
            body(tc_ctx, q_d, k_d, o_d, ns, t, c, l)
        else:
            pre = pre_pools = None
            if variant == "v3" and V3_KHOIST:
                pre, pre_pools = _v3_k_prelude(tc_ctx, k_d, ns, t, c, l)
            elif variant == "veng":
                pre, pre_pools = _v3_eng_prelude(tc_ctx, q_d, k_d, ns, t, c, l)
            elif variant == "pair":
                pre, pre_pools = _pair_prelude(tc_ctx, q_d, k_d, ns, t, c, l)
            # PE body far exceeds one IRAM block; hint the back-edge prefetch.
            with tc_ctx.For_i(
                0, repeat, 1, hint_engines=(mybir.EngineType.PE,)
            ):
                if pre is not None and variant == "v3":
                    _body_v3(tc_ctx, q_d, k_d, o_d, ns, t, c, l, k_pre=pre)
                elif pre is not None and variant == "pair":
                    _body_pair(tc_ctx, q_d, k_d, o_d, ns, t, c, l, pre=pre)
                elif pre is not None:
                    _body_veng(tc_ctx, q_d, k_d, o_d, ns, t, c, l, pre=pre)
                else:
                    body(tc_ctx, q_d, k_d, o_d, ns, t, c, l)
            if pre_pools is not None:
                for p_ in reversed(pre_pools):
                    p_.release()
    nc.compile()
    _NC_CACHE[key] = nc
    return nc


def _body_fp16(tc_ctx, q_d, k_d, o_d, ns, t, c, l):
    """fp16 compute path: q/k cast to fp16 during the SWDGE load; fp16
    matmuls for scores; fp16 multiply + pairwise-tree reduce for phase 2.
    Softmax itself stays fp32 (PSUM accumulation is fp32)."""
    from contextlib import ExitStack

    nc = tc_ctx.nc
    f32 = mybir.dt.float32
    f16 = mybir.dt.float16
    tc_sz = t * c
    F = tc_sz // P

    stack = ExitStack()
    qpool = stack.enter_context(tc_ctx.tile_pool(name="qpool", bufs=3))
    kpool = stack.enter_context(tc_ctx.tile_pool(name="kpool", bufs=2))
    small = stack.enter_context(tc_ctx.tile_pool(name="small", bufs=4))
    opool = stack.enter_context(tc_ctx.tile_pool(name="opool", bufs=2))
    pscore = stack.enter_context(tc_ctx.tile_pool(name="pscore", bufs=2, space="PSUM"))

    G = 4  # q load chunks per sample (phase-1 overlaps the load)
    Fc = F // G

    for i in range(ns):
        # ---- loads (SWDGE casts f32 -> fp16 in the DMA), chunked ----
        k_t = kpool.tile([P, F], f16)
        nc.gpsimd.dma_start(
            out=k_t[:], in_=_flat_sample_kc(k_d.ap()[:, i, :], t, c, F)
        )

        q_src = q_d.ap()[i].rearrange("t c l -> (t c l)").rearrange(
            "(p x) -> p x", p=P
        )
        q_t = qpool.tile([P, F * l], f16)
        for g in range(G):
            nc.gpsimd.dma_start(
                out=q_t[:, g * Fc * l : (g + 1) * Fc * l],
                in_=q_src[:, g * Fc * l : (g + 1) * Fc * l],
            )

        # ---- phase 1: scores[l] = sum_tc q*k  (PE fp16, PSUM fp32) ----
        q3 = q_t[:].rearrange("p (f l) -> p f l", l=l)
        ps = pscore.tile([1, l], f32)
        for f in range(F):
            nc.tensor.matmul(
                ps[:],
                lhsT=k_t[:, f : f + 1],
                rhs=q3[:, f, :],
                start=(f == 0),
                stop=(f == F - 1),
            )

        # ---- softmax on [1, l] (fp32) ----
        negmax = small.tile([1, 1], f32)
        nc.vector.tensor_reduce(
            out=negmax[:], in_=ps[:], axis=mybir.AxisListType.X,
            op=mybir.AluOpType.max, negate=True,
        )
        exps16 = small.tile([1, l], f16)
        sumexp = small.tile([1, 1], f32)
        nc.scalar.activation(
            out=exps16[:], in_=ps[:], func=mybir.ActivationFunctionType.Exp,
            bias=negmax[:], scale=1.0, accum_out=sumexp[:],
        )
        # Deferred normalization: multiply by exp now, scale by 1/sum at the
        # end (keeps the reciprocal off the critical path).
        rsum = small.tile([1, 1], f32)
        nc.vector.reciprocal(out=rsum[:], in_=sumexp[:])
        rrep = small.tile([P, 1], f32)
        nc.gpsimd.partition_broadcast(rrep[:], rsum[:])

        # broadcast exp to all partitions
        srep = small.tile([P, l], f16)
        nc.gpsimd.partition_broadcast(srep[:], exps16[:])

        # ---- phase 2: q *= s, then pairwise tree-sum over l ----
        s_b = srep[:].unsqueeze(1).to_broadcast([P, F, l])
        nc.vector.tensor_tensor(out=q3, in0=q3, in1=s_b, op=mybir.AluOpType.mult)
        hh = l // 2
        while hh >= 2:
            nc.vector.tensor_tensor(
                out=q3[:, :, 0:hh],
                in0=q3[:, :, 0:hh],
                in1=q3[:, :, hh : 2 * hh],
                op=mybir.AluOpType.add,
            )
            hh //= 2
        ored = opool.tile([P, F], f32)
        nc.vector.tensor_tensor(
            out=ored[:],
            in0=q3[:, :, 0],
            in1=q3[:, :, 1],
            op=mybir.AluOpType.add,
        )
        # deferred softmax normalization
        nc.vector.tensor_scalar_mul(out=ored[:], in0=ored[:], scalar1=rrep[:])

        # ---- store ----
        nc.sync.dma_start(
            out=_flat_sample_kc(o_d.ap()[:, i, :], t, c, F), in_=ored[:]
        )

    stack.close()


def _flat_sample_kc(ap2d, t, c, F):
    """[t, c] AP -> AP iterating flat tc grouped as P partitions x F.

    Returned AP may be 3-dim; DMA matches flat element order, so it pairs
    with a [P, F] SBUF tile.
    """
    if c % F == 0:
        hh = c // F
        return ap2d.rearrange("t (hh f) -> t hh f", hh=hh)
    else:
        assert F % c == 0
        g = F // c  # whole t-rows per partition
        return ap2d.rearrange("(p g) c -> p (g c)", g=g)


def _body(tc_ctx, q_d, k_d, o_d, ns, t, c, l):
    from contextlib import ExitStack

    nc = tc_ctx.nc
    f32 = mybir.dt.float32
    tc_sz = t * c
    F = tc_sz // P

    stack = ExitStack()
    qpool = stack.enter_context(tc_ctx.tile_pool(name="qpool", bufs=3))
    kpool = stack.enter_context(tc_ctx.tile_pool(name="kpool", bufs=2))
    small = stack.enter_context(tc_ctx.tile_pool(name="small", bufs=4))
    opool = stack.enter_context(tc_ctx.tile_pool(name="opool", bufs=2))
    pscore = stack.enter_context(tc_ctx.tile_pool(name="pscore", bufs=2, space="PSUM"))

    for i in range(ns):
        # ---- loads ----
        q_t = qpool.tile([P, F * l], f32)
        q_src = q_d.ap()[i].rearrange("t c l -> (t c l)").rearrange(
            "(p x) -> p x", p=P
        )
        nc.sync.dma_start(out=q_t[:], in_=q_src)

        k_t = kpool.tile([P, F], f32)
        nc.sync.dma_start(out=k_t[:], in_=_flat_sample_kc(k_d.ap()[:, i, :], t, c, F))

        # ---- phase 1: scores[l] = sum_tc q*k  (PE, PSUM accumulation) ----
        q3 = q_t[:].rearrange("p (f l) -> p f l", l=l)
        ps = pscore.tile([1, l], f32)
        for f in range(F):
            nc.tensor.matmul(
                ps[:],
                lhsT=k_t[:, f : f + 1],
                rhs=q3[:, f, :],
                start=(f == 0),
                stop=(f == F - 1),
            )

        # ---- softmax on [1, l] ----
        negmax = small.tile([1, 1], f32)
        nc.vector.tensor_reduce(
            out=negmax[:], in_=ps[:], axis=mybir.AxisListType.X,
            op=mybir.AluOpType.max, negate=True,
        )
        exps = small.tile([1, l], f32)
        sumexp = small.tile([1, 1], f32)
        nc.scalar.activation(
            out=exps[:], in_=ps[:], func=mybir.ActivationFunctionType.Exp,
            bias=negmax[:], scale=1.0, accum_out=sumexp[:],
        )
        rsum = small.tile([1, 1], f32)
        nc.vector.reciprocal(out=rsum[:], in_=sumexp[:])
        srow = small.tile([1, l], f32)
        nc.vector.tensor_scalar_mul(out=srow[:], in0=exps[:], scalar1=rsum[:])

        # broadcast s to all partitions
        srep = small.tile([P, l], f32)
        nc.gpsimd.partition_broadcast(srep[:], srow[:])

        # ---- phase 2: q *= s (broadcast over f) ; reduce over l ----
        s_b = srep[:].unsqueeze(1).to_broadcast([P, F, l])
        nc.vector.tensor_tensor(
            out=q3, in0=q3, in1=s_b, op=mybir.AluOpType.mult
        )
        ored = opool.tile([P, F], f32)
        nc.vector.tensor_reduce(
            out=ored[:], in_=q3, axis=mybir.AxisListType.X,
            op=mybir.AluOpType.add,
        )

        # ---- store ----
        nc.sync.dma_start(
            out=_flat_sample_kc(o_d.ap()[:, i, :], t, c, F), in_=ored[:]
        )

    stack.close()


def _body_dmaonly(tc_ctx, q_d, k_d, o_d, ns, t, c, l):
    """Timing probe: SWDGE cast loads only, no compute."""
    from contextlib import ExitStack

    nc = tc_ctx.nc
    f32 = mybir.dt.float32
    f16 = mybir.dt.float16
    F = (t * c) // P
    stack = ExitStack()
    qpool = stack.enter_context(tc_ctx.tile_pool(name="qpool", bufs=3))
    kpool = stack.enter_context(tc_ctx.tile_pool(name="kpool", bufs=2))
    opool = stack.enter_context(tc_ctx.tile_pool(name="opool", bufs=2))
    G = 4
    Fc = F // G
    for i in range(ns):
        k_t = kpool.tile([P, F], f16)
        nc.gpsimd.dma_start(
            out=k_t[:], in_=_flat_sample_kc(k_d.ap()[:, i, :], t, c, F)
        )
        q_src = q_d.ap()[i].rearrange("t c l -> (t c l)").rearrange(
            "(p x) -> p x", p=P
        )
        q_t = qpool.tile([P, F * l], f16)
        for g in range(G):
            nc.gpsimd.dma_start(
                out=q_t[:, g * Fc * l : (g + 1) * Fc * l],
                in_=q_src[:, g * Fc * l : (g + 1) * Fc * l],
            )
        ored = opool.tile([P, F], f32)
        # touch the loaded tile so the store depends on the loads
        nc.vector.tensor_copy(ored[:], q_t[:].rearrange("p (f l) -> p f l", l=l)[:, :, 0])
        nc.sync.dma_start(
            out=_flat_sample_kc(o_d.ap()[:, i, :], t, c, F), in_=ored[:]
        )
    stack.close()


def _body_noop(tc_ctx, q_d, k_d, o_d, ns, t, c, l):
    """Calibration probe: near-empty body to measure For_i loop overhead."""
    from contextlib import ExitStack

    nc = tc_ctx.nc
    f32 = mybir.dt.float32
    F = (t * c) // P
    stack = ExitStack()
    opool = stack.enter_context(tc_ctx.tile_pool(name="opool", bufs=2))
    ored = opool.tile([P, F], f32)
    nc.vector.memset(ored[:], 0.0)
    nc.sync.dma_start(
        out=_flat_sample_kc(o_d.ap()[:, 0, :], t, c, F), in_=ored[:]
    )
    stack.close()


def _body_dmahw(tc_ctx, q_d, k_d, o_d, ns, t, c, l):
    """Timing probe: HWDGE f32 loads only, no cast/compute."""
    from contextlib import ExitStack

    nc = tc_ctx.nc
    f32 = mybir.dt.float32
    F = (t * c) // P
    stack = ExitStack()
    q32pool = stack.enter_context(tc_ctx.tile_pool(name="q32pool", bufs=3))
    opool = stack.enter_context(tc_ctx.tile_pool(name="opool", bufs=2))
    G = G_LOADS
    Fc = F // G
    for i in range(ns):
        q_src = q_d.ap()[i].rearrange("t c l -> (t c l)").rearrange(
            "(p x) -> p x", p=P
        )
        q32 = q32pool.tile([P, F * l], f32)
        for g in range(G):
            sl = slice(g * Fc * l, (g + 1) * Fc * l)
            eng = nc.scalar if (DUAL_RING and g % 2) else nc.sync
            eng.dma_start(out=q32[:, sl], in_=q_src[:, sl])
        ored = opool.tile([P, F], f32)
        nc.vector.tensor_copy(
            ored[:], q32[:].rearrange("p (f l) -> p f l", l=l)[:, :, 0]
        )
        nc.scalar.dma_start(
            out=_flat_sample_kc(o_d.ap()[:, i, :], t, c, F), in_=ored[:]
        )
    stack.close()


RINGS = ("sp", "act")  # per-chunk ring rotation for the dmax probe

# ---- v6 knobs ----
V6_G = 4  # q chunks per sample
V6_Q32BUFS = 10  # (unused with sample staging; kept for cache key)
V6_Q16BUFS = 4  # per-sample fp16 q tiles
V6_CASTSPLIT_LAST = 1  # sub-split of the LAST chunk's cast (tail pipelining)
V6_POOL_CHUNKS = 0  # chunks per sample cast on Pool (gpsimd) instead of ACT
V6_POOL_F = 0  # trailing f rows of phase-2 computed on Pool instead of DVE
V6_BCAST = "pe"  # 'gp' partition_broadcast | 'pe' matmul ones + ACT copy
V6_STORE = "mix"  # 'mix' SP ring + last on SWDGE | 'sp' | 'gp'
V6_QAHEAD = 2  # samples of q DMA emitted ahead
V6_SCHED = "a"  # emission order: 'a' casts-then-mms | 'b' mms-then-split-casts | 'c' split-casts around mms
V6_SMUL = "act"  # final 1/sumexp scale on 'act' (frees DVE tail) or 'dve'
V6_HIPRI_SM = False  # emit softmax smalls at high scheduler priority
V6_PROBE = "full"  # 'full' | 'nop2' | 'nosm' | 'nomm' (non-full = WRONG outputs)
# Normalize softmax weights up-front ('up'): snorm = exp*rsum on ACT before the
# PE broadcast, so phase-2 needs no rrep/final scale — and with V6_BCAST='pe'
# + V6_STORE='mix' the steady state issues NO gpsimd ops at all. GpSimd and
# DVE 2-port ops (every phase-2 TT) arbitrate an exclusive SBUF port lock, so
# any Q7 work (SWDGE descriptor-gen, partition_broadcast) mutually blocks
# phase-2. 'defer' = old deferred-normalization path.
V6_NORM = "up"
V6_ROWS = [44, 44, 32, 8]  # uneven f-rows per chunk (small tail chunk shortens
# the last-DMA -> cast -> MM-tail -> softmax chain); None -> F//V6_G uniform
V6_PSBUFS = 2  # score PSUM banks
V6_DIV = False  # DVE tensor_scalar divide is not a valid ISA op; keep recip+scale


V6_PAIR = "actdve"  # engine pair for the 'pair' probe


def _pair_prelude(tc_ctx, q_d, k_d, ns, t, c, l):
    """Preload tiles once so the repeat loop runs dependency-free engine work."""
    nc = tc_ctx.nc
    f32 = mybir.dt.float32
    f16 = mybir.dt.float16
    F = (t * c) // P
    pool = tc_ctx.alloc_tile_pool(name="prpool", bufs=1)
    pre = {}
    q_src = q_d.ap()[0].rearrange("t c l -> (t c l)").rearrange("(p x) -> p x", p=P)
    q32 = pool.tile([P, F * l], f32, tag="q32", name="pq32")
    nc.sync.dma_start(out=q32[:], in_=q_src)
    qa = pool.tile([P, F * l], f16, tag="qa", name="pqa")
    nc.scalar.copy(out=qa[:], in_=q32[:])
    qb = pool.tile([P, F * l], f16, tag="qb", name="pqb")
    nc.scalar.copy(out=qb[:], in_=q32[:])
    srep = pool.tile([P, l], f16, tag="sr", name="psr")
    nc.vector.memset(srep[:], 0.01)
    ored = pool.tile([P, F], f32, tag="or", name="por")
    nc.vector.memset(ored[:], 0.0)
    k_t = pool.tile([P, F], f16, tag="kt", name="pkt")
    nc.gpsimd.dma_start(out=k_t[:], in_=_flat_sample_kc(k_d.ap()[:, 0, :], t, c, F))
    q32pool = tc_ctx.alloc_tile_pool(name="prq32c", bufs=V6_Q32BUFS)
    psp = tc_ctx.alloc_tile_pool(name="prpsum", bufs=2, space="PSUM")
    small2 = tc_ctx.alloc_tile_pool(name="prsmall", bufs=4)
    pre.update(
        q32=q32, qa=qa, qb=qb, srep=srep, ored=ored,
        k_t=k_t, q32pool=q32pool, psp=psp, small2=small2,
    )
    return pre, (pool, q32pool, psp, small2)


def _body_pair(tc_ctx, q_d, k_d, o_d, ns, t, c, l, pre=None):
    """Engine-interference probe: combinations of independent per-engine work
    matching v6's shape, selected by substrings in V6_PAIR (WRONG outputs).
    'dma' = real q chunk loads (rotating bufs); 'act' = chunk casts;
    'dve' = phase-2 mult+tree; 'mm' = 128 matmuls/sample; 'sm' = softmax
    chain on its own tiles. No cross-stage data deps (except dma->act when
    both present, matching v6)."""
    nc = tc_ctx.nc
    f32 = mybir.dt.float32
    F = (t * c) // P
    G = V6_G
    Fc = F // G
    q32, qa, qb, srep, ored = (
        pre["q32"], pre["qa"], pre["qb"], pre["srep"], pre["ored"]
    )
    k_t, psp, small2 = pre["k_t"], pre["psp"], pre["small2"]
    which = V6_PAIR
    q32pool = pre["q32pool"]
    for i in range(ns):
        chunks = []
        if "dma" in which:
            q_src = q_d.ap()[i].rearrange("t c l -> (t c l)").rearrange(
                "(p x) -> p x", p=P
            )
            for g in range(G):
                ch = q32pool.tile(
                    [P, Fc * l], f32, tag="q32c", name=f"pr32_{i}_{g}"
                )
                nc.sync.dma_start(
                    out=ch[:], in_=q_src[:, g * Fc * l : (g + 1) * Fc * l]
                )
                chunks.append(ch)
        if "act" in which:
            for g in range(G):
                sl = slice(g * Fc * l, (g + 1) * Fc * l)
                src = chunks[g][:] if chunks else q32[:, sl]
                nc.scalar.copy(out=qa[:, sl], in_=src)
        if "mm" in which:
            q3a = qa[:].rearrange("p (f l) -> p f l", l=l)
            ps = psp.tile([1, l], f32, tag="ps", name=f"prps{i}")
            for f in range(F):
                nc.tensor.matmul(
                    ps[:], lhsT=k_t[:, f : f + 1], rhs=q3a[:, f, :],
                    start=(f == 0), stop=(f == F - 1),
                )
            if "sm" in which:
                negmax = small2.tile([1, 1], f32, tag="negmax")
                nc.vector.tensor_reduce(
                    out=negmax[:], in_=ps[:], axis=mybir.AxisListType.X,
                    op=mybir.AluOpType.max, negate=True,
                )
                exps16 = small2.tile([1, l], mybir.dt.float16, tag="exps")
                sumexp = small2.tile([1, 1], f32, tag="sumexp")
                nc.scalar.activation(
                    out=exps16[:], in_=ps[:],
                    func=mybir.ActivationFunctionType.Exp,
                    bias=negmax[:], scale=1.0, accum_out=sumexp[:],
                )
        if "dve" in which:
            q3 = qb[:].rearrange("p (f l) -> p f l", l=l)
            s_b = srep[:].unsqueeze(1).to_broadcast([P, F, l])
            nc.vector.tensor_tensor(
                out=q3, in0=q3, in1=s_b, op=mybir.AluOpType.mult
            )
            hh = l // 2
            while hh >= 2:
                nc.vector.tensor_tensor(
                    out=q3[:, :, 0:hh], in0=q3[:, :, 0:hh],
                    in1=q3[:, :, hh : 2 * hh], op=mybir.AluOpType.add,
                )
                hh //= 2
            nc.vector.tensor_tensor(
                out=ored[:], in0=q3[:, :, 0], in1=q3[:, :, 1],
                op=mybir.AluOpType.add,
            )


def _body_v6(tc_ctx, q_d, k_d, o_d, ns, t, c, l):
    """Software-pipelined rewrite: chunk-granular f32 staging keeps the SP
    HWDGE ring free of anything but q loads; ACT runs casts one sample ahead
    of each sample's exp; stores go out on the Pool SWDGE ring so they never
    block loads; PE/DVE/ACT each see a monotonic, stall-free queue."""
    from contextlib import ExitStack

    nc = tc_ctx.nc
    f32 = mybir.dt.float32
    f16 = mybir.dt.float16
    F = (t * c) // P
    rows = list(V6_ROWS) if V6_ROWS else [F // V6_G] * V6_G
    assert sum(rows) == F
    G = len(rows)
    bounds = [0]
    for r in rows:
        bounds.append(bounds[-1] + r)

    stack = ExitStack()
    # Sample-granular f32 staging: chunk DMAs land in SLICES of one big tile.
    # Chunk-granular tiles measure ~38% slower DMA (per-tile WAW completion
    # waits starve the HWDGE ring); slice-writes into a rotating sample tile
    # sustain ~443 GB/s.
    q32pool = stack.enter_context(tc_ctx.tile_pool(name="q32pool", bufs=3))
    q16pool = stack.enter_context(tc_ctx.tile_pool(name="q16pool", bufs=V6_Q16BUFS))
    kpool = stack.enter_context(tc_ctx.tile_pool(name="kpool", bufs=2))
    small = stack.enter_context(tc_ctx.tile_pool(name="small", bufs=4))
    srpool = stack.enter_context(tc_ctx.tile_pool(name="srpool", bufs=2))
    opool = stack.enter_context(tc_ctx.tile_pool(name="opool", bufs=2))
    pscore = stack.enter_context(
        tc_ctx.tile_pool(name="pscore", bufs=V6_PSBUFS, space="PSUM")
    )
    psrep = (
        stack.enter_context(tc_ctx.tile_pool(name="psrep", bufs=2, space="PSUM"))
        if V6_BCAST == "pe"
        else None
    )

    # ---- k loads upfront (Pool SWDGE, tiny) ----
    k_ts = []
    for i in range(ns):
        k_t = kpool.tile([P, F], f16, tag=f"k{i}")
        nc.gpsimd.dma_start(
            out=k_t[:], in_=_flat_sample_kc(k_d.ap()[:, i, :], t, c, F)
        )
        k_ts.append(k_t)

    ones = None
    if V6_BCAST == "pe":
        ones = kpool.tile([1, P], f16, tag="ones")
        nc.vector.memset(ones[:], 1.0)

    q16s = [None] * ns
    q32chunks = [[None] * G for _ in range(ns)]

    def emit_qdma(i):
        q_src = q_d.ap()[i].rearrange("t c l -> (t c l)").rearrange(
            "(p x) -> p x", p=P
        )
        q16s[i] = q16pool.tile([P, F * l], f16, tag="q16", name=f"q16_{i}")
        q32 = q32pool.tile([P, F * l], f32, tag="q32", name=f"q32_{i}")
        for g in range(G):
            sl = slice(bounds[g] * l, bounds[g + 1] * l)
            nc.sync.dma_start(out=q32[:, sl], in_=q_src[:, sl])
            q32chunks[i][g] = q32

    def emit_casts(i, gs=None):
        q16 = q16s[i]
        for g in gs if gs is not None else range(G):
            ch = q32chunks[i][g]
            base = bounds[g] * l
            nel = rows[g] * l
            if g < V6_POOL_CHUNKS:
                nc.gpsimd.tensor_copy(
                    q16[:, base : base + nel], ch[:, base : base + nel]
                )
            elif g == G - 1 and V6_CASTSPLIT_LAST > 1:
                sub = nel // V6_CASTSPLIT_LAST
                for s_i in range(V6_CASTSPLIT_LAST):
                    e_i = base + (s_i + 1) * sub if s_i < V6_CASTSPLIT_LAST - 1 else base + nel
                    nc.scalar.copy(
                        out=q16[:, base + s_i * sub : e_i],
                        in_=ch[:, base + s_i * sub : e_i],
                    )
            else:
                nc.scalar.copy(
                    out=q16[:, base : base + nel], in_=ch[:, base : base + nel]
                )

    def emit_mms(i):
        ps = pscore.tile([1, l], f32, tag="ps")
        if V6_PROBE == "nomm":
            nc.vector.memset(ps[:], 0.25)
            return ps
        q3 = q16s[i][:].rearrange("p (f l) -> p f l", l=l)
        for f in range(F):
            nc.tensor.matmul(
                ps[:],
                lhsT=k_ts[i][:, f : f + 1],
                rhs=q3[:, f, :],
                start=(f == 0),
                stop=(f == F - 1),
            )
        return ps

    def emit_softmax(i, ps):
        from contextlib import nullcontext

        if V6_PROBE == "nosm":
            srep = srpool.tile([P, l], f16, tag="srep")
            nc.vector.memset(srep[:], 0.01)
            rrep = small.tile([P, 1], f32, tag="rrep")
            nc.vector.memset(rrep[:], 1.0)
            return srep, rrep
        hp = tc_ctx.high_priority() if V6_HIPRI_SM else nullcontext()
        with hp:
            negmax = small.tile([1, 1], f32, tag="negmax")
            nc.vector.tensor_reduce(
                out=negmax[:], in_=ps[:], axis=mybir.AxisListType.X,
                op=mybir.AluOpType.max, negate=True,
            )
            exps16 = small.tile([1, l], f16, tag="exps")
            sumexp = small.tile([1, 1], f32, tag="sumexp")
            nc.scalar.activation(
                out=exps16[:], in_=ps[:], func=mybir.ActivationFunctionType.Exp,
                bias=negmax[:], scale=1.0, accum_out=sumexp[:],
            )
            if V6_NORM == "up":
                # normalize before the broadcast: phase-2 then needs no
                # per-partition rrep or final scale at all.
                snorm = small.tile([1, l], f16, tag="snorm")
                if V6_DIV:
                    nc.vector.tensor_scalar(
                        out=snorm[:], in0=exps16[:], scalar1=sumexp[:],
                        scalar2=None, op0=mybir.AluOpType.divide,
                    )
                else:
                    rsum = small.tile([1, 1], f32, tag="rsum")
                    nc.vector.reciprocal(out=rsum[:], in_=sumexp[:])
                    nc.scalar.activation(
                        out=snorm[:], in_=exps16[:],
                        func=mybir.ActivationFunctionType.Copy, scale=rsum[:],
                    )
                bsrc, rrep = snorm, None
            else:
                rsum = small.tile([1, 1], f32, tag="rsum")
                nc.vector.reciprocal(out=rsum[:], in_=sumexp[:])
                rrep = small.tile([P, 1], f32, tag="rrep")
                nc.gpsimd.partition_broadcast(rrep[:], rsum[:])
                bsrc = exps16
            srep = srpool.tile([P, l], f16, tag="srep")
            if V6_BCAST == "pe":
                psr = psrep.tile([P, l], f32, tag="psr")
                nc.tensor.matmul(
                    psr[:], lhsT=ones[:], rhs=bsrc[:], start=True, stop=True
                )
                nc.scalar.copy(out=srep[:], in_=psr[:])
            else:
                nc.gpsimd.partition_broadcast(srep[:], bsrc[:])
        return srep, rrep

    def _p2(eng, q3, srep, ored, f0, f1):
        """mult by srep + pairwise tree reduce over l on f rows [f0, f1)."""
        s_b = srep[:].unsqueeze(1).to_broadcast([P, f1 - f0, l])
        sl = q3[:, f0:f1, :]
        eng.tensor_tensor(out=sl, in0=sl, in1=s_b, op=mybir.AluOpType.mult)
        hh = l // 2
        while hh >= 2:
            eng.tensor_tensor(
                out=sl[:, :, 0:hh], in0=sl[:, :, 0:hh],
                in1=sl[:, :, hh : 2 * hh], op=mybir.AluOpType.add,
            )
            hh //= 2
        eng.tensor_tensor(
            out=ored[:, f0:f1], in0=sl[:, :, 0], in1=sl[:, :, 1],
            op=mybir.AluOpType.add,
        )

    def emit_phase2(i, srep, rrep):
        q3 = q16s[i][:].rearrange("p (f l) -> p f l", l=l)
        ored = opool.tile([P, F], f32, tag="ored")
        if V6_PROBE == "nop2":
            nc.vector.tensor_copy(ored[:], q3[:, :, 0])
            if rrep is not None:
                nc.vector.tensor_scalar_mul(
                    out=ored[:], in0=ored[:], scalar1=rrep[:]
                )
            return ored
        fd = F - V6_POOL_F
        if V6_POOL_F > 0:
            _p2(nc.gpsimd, q3, srep, ored, fd, F)
        _p2(nc.vector, q3, srep, ored, 0, fd)
        if V6_NORM == "up":
            pass  # srep already normalized
        elif V6_SMUL == "act":
            nc.scalar.activation(
                out=ored[:], in_=ored[:],
                func=mybir.ActivationFunctionType.Copy, scale=rrep[:],
            )
        else:
            nc.vector.tensor_scalar_mul(
                out=ored[:, 0:fd], in0=ored[:, 0:fd], scalar1=rrep[:]
            )
            if V6_POOL_F > 0:
                nc.gpsimd.tensor_scalar_mul(
                    out=ored[:, fd:F], in0=ored[:, fd:F], scalar1=rrep[:]
                )
        return ored

    def emit_store(i, ored):
        out_ap = _flat_sample_kc(o_d.ap()[:, i, :], t, c, F)
        if V6_STORE == "sp":
            nc.sync.dma_start(out=out_ap, in_=ored[:])
        elif V6_STORE == "mix":
            # First stores ride the SP ring (emitted after every q load, and
            # their data is ready before the SDMA ring drains the loads); the
            # LAST store goes SWDGE so the next iteration's q loads never
            # queue behind a wait on this iteration's final phase-2.
            if i == ns - 1:
                nc.gpsimd.dma_start(out=out_ap, in_=ored[:])
            else:
                nc.sync.dma_start(out=out_ap, in_=ored[:])
        else:
            nc.gpsimd.dma_start(out=out_ap, in_=ored[:])

    # ---- schedule ----
    # ACT queue per sample: [cast(i+1, g0), exp_i, cast(i+1, g1..)] — the exp
    # slots in after one lookahead chunk so it never head-of-line-blocks the
    # next sample's casts, and vice versa.
    for i in range(min(V6_QAHEAD, ns)):
        emit_qdma(i)
    emit_casts(0)
    oreds = [None] * ns
    for i in range(ns):
        if i + V6_QAHEAD < ns:
            emit_qdma(i + V6_QAHEAD)
        if V6_SCHED == "a":
            if i + 1 < ns:
                emit_casts(i + 1)
            ps = emit_mms(i)
            srep, rrep = emit_softmax(i, ps)
        elif V6_SCHED == "b":
            ps = emit_mms(i)
            if i + 1 < ns:
                emit_casts(i + 1, gs=[0])
            srep, rrep = emit_softmax(i, ps)
            if i + 1 < ns:
                emit_casts(i + 1, gs=list(range(1, G)))
        else:  # 'c'
            if i + 1 < ns:
                emit_casts(i + 1, gs=[0])
            ps = emit_mms(i)
            srep, rrep = emit_softmax(i, ps)
            if i + 1 < ns:
                emit_casts(i + 1, gs=list(range(1, G)))
        oreds[i] = emit_phase2(i, srep, rrep)
        if i >= 1:
            emit_store(i - 1, oreds[i - 1])
    emit_store(ns - 1, oreds[ns - 1])

    stack.close()

# ---- v3 knobs ----
V3_G = 6  # q chunks per sample
V3_RINGS = ("sp", "act", "gpcast")  # chunk ring rotation
V3_TREE = True  # pairwise tree reduce (False: single tensor_reduce)
V3_CASTSPLIT = 1  # sub-splits of each ScalarE cast chunk
V3_LOOKAHEAD = 2  # how many samples of q DMA issue to run ahead
V3_BCAST = "gp"  # softmax broadcast path: 'pe' (matmul+ACT copy) | 'gp' (partition_broadcast)
DMAX_KLOADS = False  # add k SWDGE loads to the dmax probe
V3_KHW = False  # load k via HWDGE f32 + ScalarE cast (not SWDGE cast-DMA)
V3_KMODE = "pe"  # 'swdge' | 'khw' | 'shuffle' | 'pe' (contig k_all + PE permutation matmuls)
V3_KHOIST = True  # hoist the (iteration-invariant) k pipeline out of the repeat loop
V3_STORE = "act_end"  # 'sp_inline' | 'act_end' (stores deferred to end of body on ACT ring)
V3_ENG = "all"  # veng probe: 'act' | 'pe' | 'p2' | 'dve' | 'gp' | 'all'
V3_NOK = False  # skip k loads (dmas probe bisection)
V3_PROBE = "full"  # timing probes: 'full' | 'dmas' | 'nosm' | 'nop2' (non-full = WRONG outputs)


def _v3_chunks(F):
    rows = [F // V3_G + (1 if g < F % V3_G else 0) for g in range(V3_G)]
    chunks = []
    r0 = 0
    hw_off = 0
    for g in range(V3_G):
        ring = V3_RINGS[g % len(V3_RINGS)]
        r1 = r0 + rows[g]
        if ring in ("sp", "act"):
            chunks.append((ring, r0, r1, hw_off))
            hw_off += r1 - r0
        else:
            chunks.append((ring, r0, r1, None))
        r0 = r1
    return chunks, hw_off




def _v3_eng_prelude(tc_ctx, q_d, k_d, ns, t, c, l):
    """Load everything once so the repeat loop can exercise single engines."""
    nc = tc_ctx.nc
    f32 = mybir.dt.float32
    f16 = mybir.dt.float16
    F = (t * c) // P
    pool = tc_ctx.alloc_tile_pool(name="vepool", bufs=1)
    pspool = tc_ctx.alloc_tile_pool(name="vepsum", bufs=4, space="PSUM")
    pre = {"q32": [], "q16": [], "k": [], "ps": [], "srep": [], "o": []}
    k_ts, kpools = _v3_k_prelude(tc_ctx, k_d, ns, t, c, l)
    pre["k"] = k_ts
    for i in range(ns):
        q_src = q_d.ap()[i].rearrange("t c l -> (t c l)").rearrange(
            "(p x) -> p x", p=P
        )
        q32 = pool.tile([P, F * l], f32, tag=f"q32_{i}", name=f"vq32_{i}")
        nc.sync.dma_start(out=q32[:], in_=q_src)
        q16 = pool.tile([P, F * l], f16, tag=f"q16_{i}", name=f"vq16_{i}")
        nc.scalar.copy(out=q16[:], in_=q32[:])
        ps = pspool.tile([1, l], f32, tag="ps", name=f"vps{i}", bufs=4)
        nc.vector.memset(ps[:], 0.25)
        srep = pool.tile([P, l], f16, tag=f"sr{i}", name=f"vsr{i}")
        nc.vector.memset(srep[:], 0.01)
        ored = pool.tile([P, F], f32, tag=f"o{i}", name=f"vo{i}")
        nc.vector.memset(ored[:], 0.0)
        pre["q32"].append(q32)
        pre["q16"].append(q16)
        pre["ps"].append(ps)
        pre["srep"].append(srep)
        pre["o"].append(ored)
    return pre, (pool, pspool) + kpools


def _body_veng(tc_ctx, q_d, k_d, o_d, ns, t, c, l, pre=None):
    """Engine-isolated compute probe (V3_ENG selects the work)."""
    nc = tc_ctx.nc
    f32 = mybir.dt.float32
    f16 = mybir.dt.float16
    F = (t * c) // P
    eng = V3_ENG
    small = tc_ctx.alloc_tile_pool(name="vsmall", bufs=4)

    for i in range(ns):
        q32, q16 = pre["q32"][i], pre["q16"][i]
        k_t, ps, srep, ored = pre["k"][i], pre["ps"][i], pre["srep"][i], pre["o"][i]
        q3 = q16[:].rearrange("p (f l) -> p f l", l=l)

        if eng in ("act", "all"):
            half = (F // 2) * l
            nc.scalar.copy(out=q16[:, :half], in_=q32[:, :half])
            nc.scalar.copy(out=q16[:, half:], in_=q32[:, half:])

        if eng in ("pe", "all"):
            for f in range(F):
                nc.tensor.matmul(
                    ps[:], lhsT=k_t[:, f : f + 1], rhs=q3[:, f, :],
                    start=(f == 0), stop=(f == F - 1),
                )

        if eng in ("dve", "all"):
            negmax = small.tile([1, 1], f32, tag="negmax")
            nc.vector.tensor_reduce(
                out=negmax[:], in_=ps[:], axis=mybir.AxisListType.X,
                op=mybir.AluOpType.max, negate=True,
            )
            exps16 = small.tile([1, l], f16, tag="exps")
            sumexp = small.tile([1, 1], f32, tag="sumexp")
            if eng == "all":
                nc.scalar.activation(
                    out=exps16[:], in_=ps[:],
                    func=mybir.ActivationFunctionType.Exp,
                    bias=negmax[:], scale=1.0, accum_out=sumexp[:],
                )
            else:
                nc.vector.memset(exps16[:], 0.5)
                nc.vector.memset(sumexp[:], 32.0)
            rsum = small.tile([1, 1], f32, tag="rsum")
            nc.vector.reciprocal(out=rsum[:], in_=sumexp[:])
            snorm = small.tile([1, l], f16, tag="snorm")
            nc.vector.tensor_scalar_mul(out=snorm[:], in0=exps16[:], scalar1=rsum[:])
            if eng == "all" or V3_BCAST == "gp":
                nc.gpsimd.partition_broadcast(srep[:], snorm[:])

        if eng == "gp":
            snorm = small.tile([1, l], f16, tag="snorm")
            nc.vector.memset(snorm[:], 0.5)
            nc.gpsimd.partition_broadcast(srep[:], snorm[:])

        if eng in ("p2", "dve", "all"):
            s_b = srep[:].unsqueeze(1).to_broadcast([P, F, l])
            nc.vector.tensor_tensor(out=q3, in0=q3, in1=s_b, op=mybir.AluOpType.mult)
            if V3_TREE:
                hh = l // 2
                while hh >= 2:
                    nc.vector.tensor_tensor(
                        out=q3[:, :, 0:hh], in0=q3[:, :, 0:hh],
                        in1=q3[:, :, hh : 2 * hh], op=mybir.AluOpType.add,
                    )
                    hh //= 2
                nc.vector.tensor_tensor(
                    out=ored[:], in0=q3[:, :, 0], in1=q3[:, :, 1],
                    op=mybir.AluOpType.add,
                )
            else:
                nc.vector.tensor_reduce(
                    out=ored[:], in_=q3, axis=mybir.AxisListType.X,
                    op=mybir.AluOpType.add,
                )
    small.release()


def _v3_k_prelude(tc_ctx, k_d, ns, t, c, l):
    """Iteration-invariant k pipeline, traced once before the repeat loop:
    contiguous kall load + PE permutation matmuls + ScalarE casts."""
    nc = tc_ctx.nc
    f32 = mybir.dt.float32
    f16 = mybir.dt.float16
    F = (t * c) // P
    kpool = tc_ctx.alloc_tile_pool(name="kprepool", bufs=1)
    kppool = tc_ctx.alloc_tile_pool(name="kprepsum", bufs=2, space="PSUM")

    kall = kpool.tile([t, ns * c], f32, tag="kall", name="kall")
    nc.scalar.dma_start(out=kall[:], in_=k_d.ap().rearrange("t n c -> t (n c)"))

    ei = kpool.tile([t, P], mybir.dt.int32, tag="ei", name="ei")
    nc.gpsimd.iota(out=ei[:], pattern=[[1, P]], base=0, channel_multiplier=-2)
    eperm = []
    for pb in range(2):
        e = kpool.tile([t, P], f32, tag=f"e{pb}", name=f"e{pb}")
        nc.vector.tensor_scalar(
            out=e[:], in0=ei[:], scalar1=pb, scalar2=None,
            op0=mybir.AluOpType.is_equal,
        )
        eperm.append(e)

    k_ts = []
    for i in range(ns):
        k_t = kpool.tile([P, F], f16, tag=f"k{i}", name=f"k{i}")
        kp = kppool.tile([P, F], f32, tag="kp", name=f"kp{i}")
        nc.tensor.matmul(
            kp[:], lhsT=eperm[0][:], rhs=kall[:, i * c : i * c + F],
            start=True, stop=False,
        )
        nc.tensor.matmul(
            kp[:], lhsT=eperm[1][:], rhs=kall[:, i * c + F : (i + 1) * c],
            start=False, stop=True,
        )
        nc.scalar.copy(out=k_t[:], in_=kp[:])
        k_ts.append(k_t)
    return k_ts, (kpool, kppool)


def _body_v3(tc_ctx, q_d, k_d, o_d, ns, t, c, l, k_pre=None):
    """Three-path q loads (SP/ACT HWDGE f32 + SWDGE fp16-cast), emission
    software-pipelined: q DMAs run V3_LOOKAHEAD samples ahead and ScalarE
    casts one sample ahead of each sample's softmax, so no engine queue
    ping-pongs between DMA issue / cast / exp. Softmax is normalized up
    front; broadcast via gpsimd partition_broadcast ('gp') or a PE rank-1
    matmul + ACT copy ('pe')."""
    from contextlib import ExitStack

    nc = tc_ctx.nc
    f32 = mybir.dt.float32
    f16 = mybir.dt.float16
    F = (t * c) // P
    chunks, HW = _v3_chunks(F)

    stack = ExitStack()
    q32pool = stack.enter_context(tc_ctx.tile_pool(name="q32pool", bufs=1))
    q16pool = stack.enter_context(tc_ctx.tile_pool(name="q16pool", bufs=1))
    kpool = stack.enter_context(tc_ctx.tile_pool(name="kpool", bufs=2))
    small = stack.enter_context(tc_ctx.tile_pool(name="small", bufs=4))
    srpool = stack.enter_context(tc_ctx.tile_pool(name="srpool", bufs=2))
    opool = stack.enter_context(tc_ctx.tile_pool(name="opool", bufs=2))
    pscore = stack.enter_context(
        tc_ctx.tile_pool(name="pscore", bufs=2, space="PSUM")
    )
    psrep = stack.enter_context(
        tc_ctx.tile_pool(name="psrep", bufs=2, space="PSUM")
    )
    kppool = stack.enter_context(
        tc_ctx.tile_pool(name="kppool", bufs=2, space="PSUM")
    )

    q32s = [None] * ns
    q16s = [None] * ns
    k_ts = [None] * ns
    pending_stores = []

    k32s = [None] * ns
    kall = [None]

    def emit_kall():
        # whole per-core k in one contiguous HBM load: partition a = t row a,
        # 4 KB lines (64 descriptors)
        ka = kpool.tile([t, ns * c], f32, tag="kall", name="kall")
        nc.scalar.dma_start(
            out=ka[:], in_=k_d.ap().rearrange("t n c -> t (n c)")
        )
        kall[0] = ka

    eperm = [None, None]

    def emit_eperm():
        # E_pb[a, m] = 1.0 iff m == 2a + pb  (partition-pair shuffle operands)
        ei = kpool.tile([t, P], mybir.dt.int32, tag="ei", name="ei")
        nc.gpsimd.iota(
            out=ei[:], pattern=[[1, P]], base=0, channel_multiplier=-2
        )
        for pb in range(2):
            e = kpool.tile([t, P], f32, tag=f"e{pb}", name=f"e{pb}")
            nc.vector.tensor_scalar(
                out=e[:], in0=ei[:], scalar1=pb, scalar2=None,
                op0=mybir.AluOpType.is_equal,
            )
            eperm[pb] = e

    def emit_kload(i):
        k_t = kpool.tile([P, F], f16, tag=f"k{i}", name=f"k{i}")
        if V3_KMODE == "pe":
            kp = kppool.tile([P, F], f32, tag="kp", name=f"kp{i}")
            nc.tensor.matmul(
                kp[:], lhsT=eperm[0][:], rhs=kall[0][:, i * c : i * c + F],
                start=True, stop=False,
            )
            nc.tensor.matmul(
                kp[:], lhsT=eperm[1][:], rhs=kall[0][:, i * c + F : (i + 1) * c],
                start=False, stop=True,
            )
            nc.scalar.copy(out=k_t[:], in_=kp[:])
        elif V3_KMODE == "shuffle":
            # SBUF->SBUF SWDGE cast + partition shuffle out of kall:
            # k_t[2a+pb, f] = kall[a, i*c + pb*F + f]
            nc.gpsimd.dma_start(
                out=k_t[:].rearrange("(a pb) f -> a pb f", pb=2),
                in_=kall[0][:, i * c : (i + 1) * c].rearrange(
                    "a (pb f) -> a pb f", f=F
                ),
            )
        elif V3_KMODE == "khw" or V3_KHW:
            k32 = kpool.tile([P, F], f32, tag=f"k32_{i}", name=f"k32_{i}")
            nc.sync.dma_start(
                out=k32[:], in_=_flat_sample_kc(k_d.ap()[:, i, :], t, c, F)
            )
            k32s[i] = k32
        else:
            nc.gpsimd.dma_start(
                out=k_t[:], in_=_flat_sample_kc(k_d.ap()[:, i, :], t, c, F)
            )
        k_ts[i] = k_t

    def emit_qdma(i):
        q_src = q_d.ap()[i].rearrange("t c l -> (t c l)").rearrange(
            "(p x) -> p x", p=P
        )
        q32 = q32pool.tile([P, HW * l], f32, tag=f"q32_{i}", name=f"q32_{i}")
        q16 = q16pool.tile([P, F * l], f16, tag=f"q16_{i}", name=f"q16_{i}")
        for ring, r0, r1, off in chunks:
            src = q_src[:, r0 * l : r1 * l]
            if ring == "sp":
                nc.sync.dma_start(out=q32[:, off * l : (off + r1 - r0) * l], in_=src)
            elif ring == "act":
                nc.scalar.dma_start(out=q32[:, off * l : (off + r1 - r0) * l], in_=src)
            else:
                nc.gpsimd.dma_start(out=q16[:, r0 * l : r1 * l], in_=src)
        q32s[i] = q32
        q16s[i] = q16

    def emit_casts(i):
        q16, q32 = q16s[i], q32s[i]
        for ring, r0, r1, off in chunks:
            if off is None:
                continue
            nr = r1 - r0
            sub = nr // V3_CASTSPLIT
            subs = [sub] * V3_CASTSPLIT
            subs[-1] += nr - sub * V3_CASTSPLIT
            s0 = 0
            for srws in subs:
                nc.scalar.copy(
                    out=q16[:, (r0 + s0) * l : (r0 + s0 + srws) * l],
                    in_=q32[:, (off + s0) * l : (off + s0 + srws) * l],
                )
                s0 += srws

    # prologue: all k loads, then the pipelined head
    if k_pre is not None:
        for i in range(ns):
            k_ts[i] = k_pre[i]
    elif not V3_NOK:
        if V3_KMODE in ("shuffle", "pe"):
            emit_kall()
        if V3_KMODE == "pe":
            emit_eperm()
        for i in range(ns):
            emit_kload(i)
    look = min(V3_LOOKAHEAD, ns)
    for i in range(look):
        emit_qdma(i)
    if V3_PROBE == "dmas":
        for i in range(look, ns):
            emit_qdma(i)
        for i in range(ns):
            q16, q32 = q16s[i], q32s[i]
            ored = opool.tile([P, F], f32, tag="ored", name=f"ored{i}")
            nc.vector.tensor_copy(
                ored[:], q16[:].rearrange("p (f l) -> p f l", l=l)[:, :, 0]
            )
            nc.vector.tensor_tensor(
                out=ored[:, :HW], in0=ored[:, :HW],
                in1=q32[:].rearrange("p (f l) -> p f l", l=l)[:, :, 0],
                op=mybir.AluOpType.add,
            )
            nc.sync.dma_start(
                out=_flat_sample_kc(o_d.ap()[:, i, :], t, c, F), in_=ored[:]
            )
        stack.close()
        return

    if V3_BCAST == "pe":
        ones = small.tile([1, P], f16, tag="ones")
        nc.vector.memset(ones[:], 1.0)
    if (V3_KMODE == "khw" or V3_KHW) and V3_KMODE != "shuffle" and not V3_NOK:
        for i in range(ns):
            nc.scalar.copy(out=k_ts[i][:], in_=k32s[i][:])
    emit_casts(0)

    for i in range(ns):
        if i + look < ns:
            emit_qdma(i + look)
        if i + 1 < ns:
            emit_casts(i + 1)

        q16, q32, k_t = q16s[i], q32s[i], k_ts[i]
        # phase 1
        q3 = q16[:].rearrange("p (f l) -> p f l", l=l)
        ps = pscore.tile([1, l], f32, tag="ps", name=f"ps{i}")
        for f in range(F):
            nc.tensor.matmul(
                ps[:],
                lhsT=k_t[:, f : f + 1],
                rhs=q3[:, f, :],
                start=(f == 0),
                stop=(f == F - 1),
            )

        if V3_PROBE == "nosm":
            srep = srpool.tile([P, l], f16, tag="srep", name=f"srep{i}")
            nc.vector.memset(srep[:], 0.01)
        else:
            # softmax (normalized up front)
            negmax = small.tile([1, 1], f32, tag="negmax")
            nc.vector.tensor_reduce(
                out=negmax[:], in_=ps[:], axis=mybir.AxisListType.X,
                op=mybir.AluOpType.max, negate=True,
            )
            exps16 = small.tile([1, l], f16, tag="exps")
            sumexp = small.tile([1, 1], f32, tag="sumexp")
            nc.scalar.activation(
                out=exps16[:], in_=ps[:], func=mybir.ActivationFunctionType.Exp,
                bias=negmax[:], scale=1.0, accum_out=sumexp[:],
            )
            rsum = small.tile([1, 1], f32, tag="rsum")
            nc.vector.reciprocal(out=rsum[:], in_=sumexp[:])
            snorm = small.tile([1, l], f16, tag="snorm")
            nc.vector.tensor_scalar_mul(out=snorm[:], in0=exps16[:], scalar1=rsum[:])
            srep = srpool.tile([P, l], f16, tag="srep", name=f"srep{i}")
            if V3_BCAST == "pe":
                psr = psrep.tile([P, l], f32, tag="psr", name=f"psr{i}")
                nc.tensor.matmul(
                    psr[:], lhsT=ones[:], rhs=snorm[:], start=True, stop=True
                )
                nc.scalar.copy(out=srep[:], in_=psr[:])
            else:
                nc.gpsimd.partition_broadcast(srep[:], snorm[:])

        # phase 2
        ored = opool.tile([P, F], f32, tag="ored", name=f"ored{i}")
        if V3_PROBE == "nop2":
            nc.vector.tensor_copy(ored[:], q3[:, :, 0])
            nc.vector.tensor_scalar_mul(
                out=ored[:], in0=ored[:], scalar1=srep[:, 0:1]
            )
        else:
            s_b = srep[:].unsqueeze(1).to_broadcast([P, F, l])
            nc.vector.tensor_tensor(out=q3, in0=q3, in1=s_b, op=mybir.AluOpType.mult)
            if V3_TREE:
                hh = l // 2
                while hh >= 2:
                    nc.vector.tensor_tensor(
                        out=q3[:, :, 0:hh],
                        in0=q3[:, :, 0:hh],
                        in1=q3[:, :, hh : 2 * hh],
                        op=mybir.AluOpType.add,
                    )
                    hh //= 2
                nc.vector.tensor_tensor(
                    out=ored[:], in0=q3[:, :, 0], in1=q3[:, :, 1],
                    op=mybir.AluOpType.add,
                )
            else:
                nc.vector.tensor_reduce(
                    out=ored[:], in_=q3, axis=mybir.AxisListType.X,
                    op=mybir.AluOpType.add,
                )

        # store
        out_ap = _flat_sample_kc(o_d.ap()[:, i, :], t, c, F)
        if V3_STORE == "act_end":
            pending_stores.append((out_ap, ored[:]))
        else:
            nc.sync.dma_start(out=out_ap, in_=ored[:])

    for ap_o, t_o in pending_stores:
        nc.scalar.dma_start(out=ap_o, in_=t_o)

    stack.close()


def _body_dmax(tc_ctx, q_d, k_d, o_d, ns, t, c, l):
    """Timing probe: q loads spread across rings per RINGS rotation.
    'sp'/'act' = HWDGE f32; 'gp' = SWDGE f32; 'gpcast' = SWDGE f32->fp16."""
    from contextlib import ExitStack

    nc = tc_ctx.nc
    f32 = mybir.dt.float32
    f16 = mybir.dt.float16
    F = (t * c) // P
    stack = ExitStack()
    q32pool = stack.enter_context(tc_ctx.tile_pool(name="q32pool", bufs=2))
    q16pool = stack.enter_context(tc_ctx.tile_pool(name="q16pool", bufs=2))
    opool = stack.enter_context(tc_ctx.tile_pool(name="opool", bufs=2))
    kpool = stack.enter_context(tc_ctx.tile_pool(name="kpool", bufs=2))
    if DMAX_KLOADS:
        for i in range(ns):
            k_t = kpool.tile([P, F], f16, tag=f"k{i}", name=f"k{i}")
            nc.gpsimd.dma_start(
                out=k_t[:], in_=_flat_sample_kc(k_d.ap()[:, i, :], t, c, F)
            )
    G = G_LOADS
    rows = [F // G + (1 if g < F % G else 0) for g in range(G)]
    bounds = [0]
    for r in rows:
        bounds.append(bounds[-1] + r)
    for i in range(ns):
        q_src = q_d.ap()[i].rearrange("t c l -> (t c l)").rearrange(
            "(p x) -> p x", p=P
        )
        q32 = q32pool.tile([P, F * l], f32)
        q16 = q16pool.tile([P, F * l], f16)
        touch = []
        for g in range(G):
            sl = slice(bounds[g] * l, bounds[g + 1] * l)
            ring = RINGS[g % len(RINGS)]
            if ring == "sp":
                nc.sync.dma_start(out=q32[:, sl], in_=q_src[:, sl])
                touch.append(q32)
            elif ring == "act":
                nc.scalar.dma_start(out=q32[:, sl], in_=q_src[:, sl])
                touch.append(q32)
            elif ring == "gp":
                nc.gpsimd.dma_start(out=q32[:, sl], in_=q_src[:, sl])
                touch.append(q32)
            elif ring == "gpcast":
                nc.gpsimd.dma_start(out=q16[:, sl], in_=q_src[:, sl])
                touch.append(q16)
        ored = opool.tile([P, F], f32)
        srcs = {id(x): x for x in touch}
        for j, x in enumerate(srcs.values()):
            if j == 0:
                nc.vector.tensor_copy(
                    ored[:], x[:].rearrange("p (f l) -> p f l", l=l)[:, :, 0]
                )
            else:
                nc.vector.tensor_tensor(
                    out=ored[:], in0=ored[:],
                    in1=x[:].rearrange("p (f l) -> p f l", l=l)[:, :, 0],
                    op=mybir.AluOpType.add,
                )
        nc.sync.dma_start(
            out=_flat_sample_kc(o_d.ap()[:, i, :], t, c, F), in_=ored[:]
        )
    stack.close()


def _body_fp16hw(tc_ctx, q_d, k_d, o_d, ns, t, c, l):
    """Like _body_fp16 but loads q as f32 via HWDGE (full DMA rate) and casts
    f32 -> fp16 on the (otherwise idle) ScalarE."""
    from contextlib import ExitStack

    nc = tc_ctx.nc
    f32 = mybir.dt.float32
    f16 = mybir.dt.float16
    F = (t * c) // P

    stack = ExitStack()
    q32pool = stack.enter_context(tc_ctx.tile_pool(name="q32pool", bufs=3))
    qpool = stack.enter_context(tc_ctx.tile_pool(name="qpool", bufs=QBUFS))
    kpool = stack.enter_context(tc_ctx.tile_pool(name="kpool", bufs=2))
    small = stack.enter_context(tc_ctx.tile_pool(name="small", bufs=4))
    opool = stack.enter_context(tc_ctx.tile_pool(name="opool", bufs=4))
    pscore = stack.enter_context(tc_ctx.tile_pool(name="pscore", bufs=2, space="PSUM"))

    rows = CHUNK_ROWS if CHUNK_ROWS else [F // G_LOADS] * G_LOADS
    assert sum(rows) == F
    bounds = [0]
    for r in rows:
        bounds.append(bounds[-1] + r)

    # All k loads upfront (SWDGE, tiny) so nothing later blocks them.
    k_ts = []
    for i in range(ns):
        k_t = kpool.tile([P, F], f16, tag=f"k{i}")
        nc.gpsimd.dma_start(
            out=k_t[:], in_=_flat_sample_kc(k_d.ap()[:, i, :], t, c, F)
        )
        k_ts.append(k_t)

    # Stores are emitted two samples late: a store's semaphore wait (on the
    # phase-2 result) must never block later q-load issues on the SP ring.
    pending_stores = []

    def flush_store():
        ap_out, tile_in = pending_stores.pop(0)
        nc.sync.dma_start(out=ap_out, in_=tile_in)

    for i in range(ns):
        k_t = k_ts[i]
        q_src = q_d.ap()[i].rearrange("t c l -> (t c l)").rearrange(
            "(p x) -> p x", p=P
        )
        q32 = q32pool.tile([P, F * l], f32)
        q_t = qpool.tile([P, F * l], f16)
        for g in range(len(rows)):
            sl = slice(bounds[g] * l, bounds[g + 1] * l)
            eng = nc.scalar if (DUAL_RING and g % 2) else nc.sync
            eng.dma_start(out=q32[:, sl], in_=q_src[:, sl])
            if g == len(rows) - 1 and CASTSPLIT > 1:
                # tail chunk: sub-split the cast so its matmuls pipeline
                # behind sub-casts instead of one long cast
                sub = rows[g] // CASTSPLIT
                for s_i in range(CASTSPLIT):
                    ss = slice(
                        (bounds[g] + s_i * sub) * l,
                        (bounds[g] + (s_i + 1) * sub) * l,
                    )
                    nc.scalar.copy(out=q_t[:, ss], in_=q32[:, ss])
            else:
                nc.scalar.copy(out=q_t[:, sl], in_=q32[:, sl])

        # ---- phase 1 ----
        q3 = q_t[:].rearrange("p (f l) -> p f l", l=l)
        ps = pscore.tile([1, l], f32)
        for f in range(F):
            nc.tensor.matmul(
                ps[:],
                lhsT=k_t[:, f : f + 1],
                rhs=q3[:, f, :],
                start=(f == 0),
                stop=(f == F - 1),
            )

        # ---- softmax (deferred normalization) ----
        negmax = small.tile([1, 1], f32)
        nc.vector.tensor_reduce(
            out=negmax[:], in_=ps[:], axis=mybir.AxisListType.X,
            op=mybir.AluOpType.max, negate=True,
        )
        exps16 = small.tile([1, l], f16)
        sumexp = small.tile([1, 1], f32)
        nc.scalar.activation(
            out=exps16[:], in_=ps[:], func=mybir.ActivationFunctionType.Exp,
            bias=negmax[:], scale=1.0, accum_out=sumexp[:],
        )
        # srep broadcast first (gates the phase-2 multiply); rrep is only
        # needed at the final scale, so it goes second on the POOL stream.
        srep = small.tile([P, l], f16)
        nc.gpsimd.partition_broadcast(srep[:], exps16[:])
        rsum = small.tile([1, 1], f32)
        nc.vector.reciprocal(out=rsum[:], in_=sumexp[:])
        rrep = small.tile([P, 1], f32)
        nc.gpsimd.partition_broadcast(rrep[:], rsum[:])

        # ---- phase 2 ----
        ored = opool.tile([P, F], f32)
        if MULPROBE == 2:
            # timing probe: no phase-2 at all (WRONG outputs)
            nc.vector.tensor_copy(ored[:], q3[:, :, 0])
        else:
            if not MULPROBE:
                s_b = srep[:].unsqueeze(1).to_broadcast([P, F, l])
                nc.vector.tensor_tensor(
                    out=q3, in0=q3, in1=s_b, op=mybir.AluOpType.mult
                )
            hh = l // 2
            while hh >= 2:
                nc.vector.tensor_tensor(
                    out=q3[:, :, 0:hh],
                    in0=q3[:, :, 0:hh],
                    in1=q3[:, :, hh : 2 * hh],
                    op=mybir.AluOpType.add,
                )
                hh //= 2
            nc.vector.tensor_tensor(
                out=ored[:], in0=q3[:, :, 0], in1=q3[:, :, 1],
                op=mybir.AluOpType.add,
            )
        out_ap = _flat_sample_kc(o_d.ap()[:, i, :], t, c, F)
        if STORESPLIT > 1:
            half = F // 2
            nc.vector.tensor_scalar_mul(
                out=ored[:, :half], in0=ored[:, :half], scalar1=rrep[:]
            )
            pending_stores.append((out_ap[:, :, :half], ored[:, :half]))
            nc.vector.tensor_scalar_mul(
                out=ored[:, half:], in0=ored[:, half:], scalar1=rrep[:]
            )
            pending_stores.append((out_ap[:, :, half:], ored[:, half:]))
        else:
            nc.vector.tensor_scalar_mul(out=ored[:], in0=ored[:], scalar1=rrep[:])
            pending_stores.append((out_ap, ored[:]))
        while len(pending_stores) > 2 * STORESPLIT:
            flush_store()

    while pending_stores:
        flush_store()

    stack.close()


def run(query, key, repeat=1, variant=None, **spmd_kwargs):
    query = np.ascontiguousarray(np.asarray(query, dtype=np.float32))
    key = np.asarray(key, dtype=np.float32)
    n, t, c, l = query.shape
    ncores = NCORES
    ns = n // ncores
    nc = build_nc(ns, t, c, l, variant=variant, repeat=repeat)

    in_maps = []
    for i in range(ncores):
        in_maps.append(
            {
                "q": np.ascontiguousarray(query[i * ns : (i + 1) * ns]),
                "k": np.ascontiguousarray(key[:, i * ns : (i + 1) * ns, :]),
            }
        )
    res = bass_utils.run_bass_kernel_spmd(
        nc, in_maps, core_ids=list(range(ncores)), **spmd_kwargs
    )
    out = np.empty((t, n, c), dtype=np.float32)
    for i in range(ncores):
        out[:, i * ns : (i + 1) * ns, :] = res.results[i]["o"]
    return out, res


def kernel(**inputs):
    out, _ = run(inputs["query"], inputs["key"])
    return out

